# revision 9
# baseline (speedup 1.0000x reference)
"""BitnetMLP on 8 TRN2 NeuronCores — Megatron tensor-parallel over the
intermediate dim I, with exact integer arithmetic on the TensorEngine.

Math: activation fake-quant makes activations exact int8 values and weight
fake-quant makes weights exact ternary values. Both are exactly representable
in bf16/fp8e4, and PSUM accumulates in f32, so every matmul is computed as an
exact integer matmul at full bf16 speed; per-token / per-tensor dequant scales
are applied to the f32 partial sums afterward.

Sharding (per core r of 8):
  w_gate/w_up: I-column shard (1024 of 8192)  -> h^T shard [I_sh=1024, T]
  w_down:      I-row shard                    -> partial y, ReduceScatter(add)
  per-token RMS var and abs-max stats over the full I: AllReduce add / max.

Layouts are feature-major (host pre-transposes x and the weights so the
contract dim lands on SBUF partitions; no on-device transposes of x/w/h).
"""
import numpy as np

N_CORES = 8
B, S, H, I = 2, 2048, 2048, 8192
T = B * S                      # 4096 tokens
ISH = I // N_CORES             # 1024  I shard per core
TG = 512                       # tokens per group
NG = T // TG                   # 8 groups
KC = H // 128                  # 16 contract chunks for gate/up
IC = ISH // 128                # 8  contract chunks for down / h^T partition chunks
NH = 2048 // 512               # 4  output col groups for down
NTC = TG // 128                # 4  token tiles per group
RS_BATCH = 2                   # groups per ReduceScatter
NB = NG // RS_BATCH            # 4 RS batches

MAGIC = float(1.5 * 2 ** 23)   # f32 round-to-nearest-even forcing constant
EPS = 1e-5
RMS_EPS = 1e-6

_CACHED = {}


def _build():
    import concourse.bass as bass
    import concourse.bacc as bacc
    import concourse.tile as tile
    import concourse.mybir as mybir
    from concourse import masks

    dt = mybir.dt
    AO = mybir.AluOpType
    AF = mybir.ActivationFunctionType
    RG = [list(range(N_CORES))]

    nc = bacc.Bacc("TRN2", target_bir_lowering=False, debug=False,
                   num_devices=N_CORES)

    xT_in = nc.dram_tensor("xT", [H, T], dt.float32, kind="ExternalInput")
    wgT_in = nc.dram_tensor("wgT", [H, ISH], dt.float32, kind="ExternalInput")
    wuT_in = nc.dram_tensor("wuT", [H, ISH], dt.float32, kind="ExternalInput")
    wdT_in = nc.dram_tensor("wdT", [ISH, 2048], dt.float32, kind="ExternalInput")
    lnw_in = nc.dram_tensor("lnw", [ISH], dt.float32, kind="ExternalInput")
    y_out = nc.dram_tensor("y_out", [T // N_CORES, 2048], dt.float32,
                           kind="ExternalOutput")

    from contextlib import ExitStack

    with tile.TileContext(nc) as tc:
        with ExitStack() as stack:
            ep = stack.enter_context
            constp = ep(tc.tile_pool(name="const", bufs=1))
            wqp = ep(tc.tile_pool(name="wq", bufs=1))
            wstage = ep(tc.tile_pool(name="wstage", bufs=3))
            xstage = ep(tc.tile_pool(name="xstage", bufs=2))
            qxp = ep(tc.tile_pool(name="qx", bufs=2))
            hbp = ep(tc.tile_pool(name="hbuf", bufs=2))
            qhp = ep(tc.tile_pool(name="qh", bufs=2))
            bcp = ep(tc.tile_pool(name="bc", bufs=1))
            smp = ep(tc.tile_pool(name="small", bufs=2))
            rowp = ep(tc.tile_pool(name="rows", bufs=4))
            evp = ep(tc.tile_pool(name="evac", bufs=2))
            ps_gu = ep(tc.tile_pool(name="ps_gu", bufs=2, space="PSUM"))
            ps_dn = ep(tc.tile_pool(name="ps_dn", bufs=2, space="PSUM"))
            ps_ss = ep(tc.tile_pool(name="ps_ss", bufs=1, space="PSUM"))
            ps_misc = ep(tc.tile_pool(name="ps_misc", bufs=1, space="PSUM"))
            dram = ep(tc.tile_pool(name="dram", bufs=1, space="DRAM"))
            dram_rs = ep(tc.tile_pool(name="dram_rs", bufs=2, space="DRAM"))
            # ---------- constants ----------
            ident = constp.tile([128, 128], dt.float32)
            masks.make_identity(nc, ident[:])
            ones_col = constp.tile([128, 1], dt.float32)   # lhsT for partition sums
            nc.vector.memset(ones_col[:], 1.0)
            ones_row = constp.tile([1, 128], dt.float32)   # lhsT for K=1 broadcasts
            nc.vector.memset(ones_row[:], 1.0)
            lnw_sb = constp.tile([128, IC], dt.float32)    # lnw[128*ic + p] at [p, ic]
            nc.sync.dma_start(lnw_sb[:], lnw_in.rearrange("(c p) -> p c", p=128)[:])
            alnw_sb = constp.tile([128, IC], dt.float32)   # |lnw|
            nc.vector.tensor_scalar(alnw_sb.bitcast(dt.uint32)[:],
                                    lnw_sb.bitcast(dt.uint32)[:],
                                    0x7FFFFFFF, None, AO.bitwise_and)

            # ---------- internal DRAM ----------
            y_partial = dram.tile([T, 2048], dt.float32)
            ss_part = dram.tile([T], dt.float32)
            ss_glob = dram.tile([T], dt.float32)
            pm_part = dram.tile([T], dt.float32)
            pm_glob = dram.tile([T], dt.float32)
            wsum_part = dram.tile([8], dt.float32)
            wsum_glob = dram.tile([8], dt.float32)
            row_bounce = dram.tile([NG, 4, TG], dt.float32)  # per-group gather bounce

            # ---------- weight abs-sum stats ----------
            wsum_row = rowp.tile([1, 8], dt.float32, tag="wsum_row")
            w_stats = [
                ("wg", wgT_in, KC, ISH),
                ("wu", wuT_in, KC, ISH),
                ("wd", wdT_in, IC, 2048),
            ]
            for wi, (wname, w_in, nchunk, wcols) in enumerate(w_stats):
                acc = smp.tile([128, 1], dt.float32, tag="wacc")
                for c in range(nchunk):
                    for cc in range(wcols // 512):
                        st = wstage.tile([128, 512], dt.float32, tag="wstage")
                        nc.sync.dma_start(st[:], w_in[c * 128:(c + 1) * 128,
                                                      cc * 512:(cc + 1) * 512])
                        red = smp.tile([128, 1], dt.float32, tag="wred")
                        nc.vector.tensor_reduce(red[:], st[:], mybir.AxisListType.X,
                                                AO.add, apply_absolute_value=True)
                        if c == 0 and cc == 0:
                            nc.vector.tensor_copy(acc[:], red[:])
                        else:
                            nc.vector.tensor_tensor(acc[:], acc[:], red[:], AO.add)
                wsum_ps = ps_misc.tile([128, 512], dt.float32, tag="misc_ps")
                nc.tensor.matmul(wsum_ps[0:1, 0:1], ones_col[:], acc[:], start=True,
                                 stop=True)
                nc.scalar.copy(wsum_row[:, wi:wi + 1], wsum_ps[0:1, 0:1])
            nc.vector.memset(wsum_row[:, 3:8], 0.0)
            nc.sync.dma_start(wsum_part.rearrange("(o f) -> o f", o=1)[:], wsum_row[:])
            nc.gpsimd.collective_compute(
                "AllReduce", AO.add, replica_groups=RG,
                ins=[wsum_part.opt()], outs=[wsum_glob.opt()])

            # scl_row layout: [sw_g, sw_u, sw_d, mg/127, mu/127, md, 0, 0]
            # sw_* = 1/clip(mean|w|, EPS); m* = clip(mean|w|, EPS)
            wsg_row = rowp.tile([1, 8], dt.float32, tag="wsg_row")
            nc.sync.dma_start(wsg_row[:], wsum_glob.rearrange("(o f) -> o f", o=1)[:])
            mean_row = rowp.tile([1, 8], dt.float32, tag="mean_row")
            nc.vector.tensor_scalar(mean_row[:, 0:3], wsg_row[:, 0:3],
                                    float(1.0 / (I * H)), EPS, AO.mult, AO.max)
            scl_row = rowp.tile([1, 8], dt.float32, tag="scl_row")
            rw = rowp.tile([1, 8], dt.float32, tag="rw_row")
            nc.vector.reciprocal(rw[:, 0:3], mean_row[:, 0:3])
            # newton: r1 = r0*(2 - m*r0)
            nt = rowp.tile([1, 8], dt.float32, tag="nt_row")
            nc.vector.tensor_tensor(nt[:, 0:3], mean_row[:, 0:3], rw[:, 0:3], AO.mult)
            nc.vector.tensor_scalar(nt[:, 0:3], nt[:, 0:3], -1.0, 2.0, AO.mult, AO.add)
            nc.vector.tensor_tensor(scl_row[:, 0:3], rw[:, 0:3], nt[:, 0:3], AO.mult)
            nc.vector.tensor_scalar(scl_row[:, 3:5], mean_row[:, 0:2],
                                    float(1.0 / 127.0), None, AO.mult)
            nc.vector.tensor_copy(scl_row[:, 5:6], mean_row[:, 2:3])
            nc.vector.memset(scl_row[:, 6:8], 0.0)
            # broadcast to all partitions
            wst_ps = ps_misc.tile([128, 512], dt.float32, tag="misc_ps")
            nc.tensor.matmul(wst_ps[:, 0:8], ones_row[:], scl_row[:], start=True,
                             stop=True)
            wstats = constp.tile([128, 8], dt.float32)
            nc.vector.tensor_copy(wstats[:], wst_ps[:, 0:8])

            # ---------- quantize weights to ternary fp8 ----------
            qwg = wqp.tile([128, KC * ISH], dt.float8e4)
            qwu = wqp.tile([128, KC * ISH], dt.float8e4)
            qwd = wqp.tile([128, IC * 2048], dt.float8e4)
            for (w_in, qw, nchunk, wcols, si) in (
                (wgT_in, qwg, KC, ISH, 0),
                (wuT_in, qwu, KC, ISH, 1),
                (wdT_in, qwd, IC, 2048, 2),
            ):
                for c in range(nchunk):
                    for cc in range(wcols // 512):
                        st = wstage.tile([128, 512], dt.float32, tag="wstage")
                        nc.sync.dma_start(st[:], w_in[c * 128:(c + 1) * 128,
                                                      cc * 512:(cc + 1) * 512])
                        t1 = wstage.tile([128, 512], dt.float32, tag="wq_t1")
                        nc.vector.tensor_scalar(t1[:], st[:], wstats[:, si:si + 1],
                                                MAGIC, AO.mult, AO.add)
                        nc.vector.tensor_scalar(t1[:], t1[:], -MAGIC, 1.0, AO.add,
                                                AO.min)
                        o0 = c * wcols + cc * 512
                        nc.vector.tensor_scalar(qw[:, o0:o0 + 512], t1[:],
                                                -1.0, None, AO.max)

            # ---------- main pipeline over token groups ----------
            for g in range(NG):
                tok0 = g * TG
                # ---- phase 1a: x abs-max over H ----
                xmax = smp.tile([128, TG], dt.float32, tag="xmax")
                for kc in range(KC):
                    st = xstage.tile([128, TG], dt.float32, tag="xs1")
                    nc.sync.dma_start(st[:], xT_in[kc * 128:(kc + 1) * 128,
                                                   tok0:tok0 + TG])
                    if kc == 0:
                        nc.vector.tensor_scalar(xmax.bitcast(dt.uint32)[:],
                                                st.bitcast(dt.uint32)[:],
                                                0x7FFFFFFF, None, AO.bitwise_and)
                    else:
                        nc.vector.tensor_scalar(st.bitcast(dt.uint32)[:],
                                                st.bitcast(dt.uint32)[:],
                                                0x7FFFFFFF, None, AO.bitwise_and)
                        nc.vector.tensor_tensor(xmax[:], xmax[:], st[:], AO.max)
                mx_nat = smp.tile([128, NTC], dt.float32, tag="mx_nat")
                for c in range(NTC):
                    tr_ps = ps_misc.tile([128, 512], dt.float32, tag="misc_ps")
                    nc.tensor.transpose(tr_ps[:, 0:128], xmax[:, c * 128:(c + 1) * 128],
                                        ident[:])
                    nc.vector.tensor_reduce(mx_nat[:, c:c + 1], tr_ps[:, 0:128],
                                            mybir.AxisListType.X, AO.max)
                # mxc = clip(max, EPS); per-token scales in [128, NTC] layout
                nc.vector.tensor_scalar(mx_nat[:], mx_nat[:], EPS, None, AO.max)
                # sx = 127/mxc  (reciprocal + newton)
                r0 = smp.tile([128, NTC], dt.float32, tag="sx_r0")
                nc.vector.reciprocal(r0[:], mx_nat[:])
                ntr = smp.tile([128, NTC], dt.float32, tag="sx_nt")
                nc.vector.tensor_tensor(ntr[:], mx_nat[:], r0[:], AO.mult)
                nc.vector.tensor_scalar(ntr[:], ntr[:], -1.0, 2.0, AO.mult, AO.add)
                sxn = smp.tile([128, NTC], dt.float32, tag="sxn")
                nc.vector.tensor_tensor(sxn[:], r0[:], ntr[:], AO.mult)
                nc.vector.tensor_scalar(sxn[:], sxn[:], 127.0, None, AO.mult)
                cgn = smp.tile([128, NTC], dt.float32, tag="cgn")
                nc.vector.tensor_scalar(cgn[:], mx_nat[:], wstats[:, 3:4], None,
                                        AO.mult)
                cun = smp.tile([128, NTC], dt.float32, tag="cun")
                nc.vector.tensor_scalar(cun[:], mx_nat[:], wstats[:, 4:5], None,
                                        AO.mult)
                # gather each to a [1, TG] row via DRAM, broadcast to [128, TG]
                sx_tile = bcp.tile([128, TG], dt.float32, tag="sx_tile")
                cg_tile = bcp.tile([128, TG], dt.float32, tag="cg_tile")
                cu_tile = bcp.tile([128, TG], dt.float32, tag="cu_tile")
                for slot, (nat, tile_) in enumerate(
                        ((sxn, sx_tile), (cgn, cg_tile), (cun, cu_tile))):
                    nc.sync.dma_start(
                        row_bounce[g, slot].rearrange("(c p) -> p c", p=128)[:],
                        nat[:])
                    row = rowp.tile([1, TG], dt.float32, tag="grow")
                    nc.sync.dma_start(
                        row[:], row_bounce[g, slot].rearrange("(o f) -> o f", o=1)[:])
                    bc_ps = ps_misc.tile([128, TG], dt.float32, tag="misc_ps")
                    nc.tensor.matmul(bc_ps[:], ones_row[:], row[:], start=True,
                                     stop=True)
                    nc.scalar.copy(tile_[:], bc_ps[:])

                # ---- phase 1b: quantize x to bf16 ints ----
                qxT = qxp.tile([128, KC * TG], dt.bfloat16, tag="qxT")
                for kc in range(KC):
                    st = xstage.tile([128, TG], dt.float32, tag="xs2")
                    nc.sync.dma_start(st[:], xT_in[kc * 128:(kc + 1) * 128,
                                                   tok0:tok0 + TG])
                    tq = xstage.tile([128, TG], dt.float32, tag="xq_t")
                    nc.vector.tensor_tensor(tq[:], st[:], sx_tile[:], AO.mult)
                    nc.vector.tensor_scalar(tq[:], tq[:], MAGIC, -MAGIC, AO.add, AO.add)
                    nc.vector.tensor_scalar(qxT[:, kc * TG:(kc + 1) * TG], tq[:],
                                            127.0, -128.0, AO.min, AO.max)

                # ---- phase 1c: gate/up matmuls + h + stats ----
                hT = hbp.tile([128, IC * TG], dt.float32, tag="hT")
                maxt = smp.tile([128, TG], dt.float32, tag="maxt")
                nc.vector.memset(maxt[:], 0.0)
                ss_ps = ps_ss.tile([1, TG], dt.float32, tag="ss_ps")
                for ic in range(IC):
                    g_ps = ps_gu.tile([128, TG], dt.float32, tag="g_ps")
                    u_ps = ps_gu.tile([128, TG], dt.float32, tag="u_ps")
                    for kc in range(KC):
                        nc.tensor.matmul(
                            g_ps[:],
                            qwg[:, kc * ISH + ic * 128: kc * ISH + (ic + 1) * 128],
                            qxT[:, kc * TG:(kc + 1) * TG],
                            start=(kc == 0), stop=(kc == KC - 1))
                    for kc in range(KC):
                        nc.tensor.matmul(
                            u_ps[:],
                            qwu[:, kc * ISH + ic * 128: kc * ISH + (ic + 1) * 128],
                            qxT[:, kc * TG:(kc + 1) * TG],
                            start=(kc == 0), stop=(kc == KC - 1))
                    gv = evp.tile([128, TG], dt.float32, tag="gv")
                    nc.vector.tensor_tensor(gv[:], g_ps[:], cg_tile[:], AO.mult)
                    sv = evp.tile([128, TG], dt.float32, tag="sv")
                    nc.scalar.activation(sv[:], gv[:], AF.Silu)
                    uv = evp.tile([128, TG], dt.float32, tag="uv")
                    nc.vector.tensor_tensor(uv[:], u_ps[:], cu_tile[:], AO.mult)
                    hslice = hT[:, ic * TG:(ic + 1) * TG]
                    nc.vector.tensor_tensor(hslice, sv[:], uv[:], AO.mult)
                    h2 = evp.tile([128, TG], dt.float32, tag="h2")
                    nc.scalar.square(h2[:], hslice)
                    nc.tensor.matmul(ss_ps[:], ones_col[:], h2[:],
                                     start=(ic == 0), stop=(ic == IC - 1))
                    ha = evp.tile([128, TG], dt.float32, tag="ha")
                    nc.vector.tensor_scalar(ha.bitcast(dt.uint32)[:],
                                            hT.bitcast(dt.uint32)[:, ic * TG:(ic + 1) * TG],
                                            0x7FFFFFFF, None, AO.bitwise_and)
                    nc.vector.scalar_tensor_tensor(maxt[:], ha[:],
                                                   alnw_sb[:, ic:ic + 1], maxt[:],
                                                   AO.mult, AO.max)
                pm_nat = smp.tile([128, NTC], dt.float32, tag="pm_nat")
                for c in range(NTC):
                    tr_ps = ps_misc.tile([128, 512], dt.float32, tag="misc_ps")
                    nc.tensor.transpose(tr_ps[:, 0:128], maxt[:, c * 128:(c + 1) * 128],
                                        ident[:])
                    nc.vector.tensor_reduce(pm_nat[:, c:c + 1], tr_ps[:, 0:128],
                                            mybir.AxisListType.X, AO.max)
                ss_row = rowp.tile([1, TG], dt.float32, tag="grow")
                nc.vector.tensor_copy(ss_row[:], ss_ps[:])
                nc.sync.dma_start(ss_part[tok0:tok0 + TG]
                                  .rearrange("(o f) -> o f", o=1)[:], ss_row[:])
                nc.sync.dma_start(pm_part[tok0:tok0 + TG]
                                  .rearrange("(c p) -> p c", p=128)[:], pm_nat[:])
                nc.gpsimd.collective_compute(
                    "AllReduce", AO.add, replica_groups=RG,
                    ins=[ss_part[tok0:tok0 + TG].opt()],
                    outs=[ss_glob[tok0:tok0 + TG].opt()])
                nc.gpsimd.collective_compute(
                    "AllReduce", AO.max, replica_groups=RG,
                    ins=[pm_part[tok0:tok0 + TG].opt()],
                    outs=[pm_glob[tok0:tok0 + TG].opt()])

                # ---- phase 2a: global per-token scales ----
                ssg = smp.tile([128, NTC], dt.float32, tag="ssg")
                nc.sync.dma_start(ssg[:], ss_glob[tok0:tok0 + TG]
                                  .rearrange("(c p) -> p c", p=128)[:])
                pmg = smp.tile([128, NTC], dt.float32, tag="pmg")
                nc.sync.dma_start(pmg[:], pm_glob[tok0:tok0 + TG]
                                  .rearrange("(c p) -> p c", p=128)[:])
                # r = 1/sqrt(ss/I + RMS_EPS)
                vr = smp.tile([128, NTC], dt.float32, tag="vr")
                nc.vector.tensor_scalar(vr[:], ssg[:], float(1.0 / I), RMS_EPS,
                                        AO.mult, AO.add)
                sq = smp.tile([128, NTC], dt.float32, tag="sq")
                nc.scalar.sqrt(sq[:], vr[:])
                rr = smp.tile([128, NTC], dt.float32, tag="rr")
                nc.vector.reciprocal(rr[:], sq[:])
                ntn = smp.tile([128, NTC], dt.float32, tag="ntn")
                nc.vector.tensor_tensor(ntn[:], sq[:], rr[:], AO.mult)
                nc.vector.tensor_scalar(ntn[:], ntn[:], -1.0, 2.0, AO.mult, AO.add)
                nc.vector.tensor_tensor(rr[:], rr[:], ntn[:], AO.mult)
                # rmc = clip(r*pmax, EPS); cd = rmc * md / 127
                rmc = smp.tile([128, NTC], dt.float32, tag="rmc")
                nc.vector.tensor_tensor(rmc[:], rr[:], pmg[:], AO.mult)
                nc.vector.tensor_scalar(rmc[:], rmc[:], EPS, None, AO.max)
                cd = smp.tile([128, NTC], dt.float32, tag="cd")
                nc.vector.tensor_scalar(cd[:], rmc[:], wstats[:, 5:6],
                                        float(1.0 / 127.0), AO.mult, AO.mult)
                # alpha = 127 * r / rmc
                ar0 = smp.tile([128, NTC], dt.float32, tag="ar0")
                nc.vector.reciprocal(ar0[:], rmc[:])
                ntn2 = smp.tile([128, NTC], dt.float32, tag="ntn2")
                nc.vector.tensor_tensor(ntn2[:], rmc[:], ar0[:], AO.mult)
                nc.vector.tensor_scalar(ntn2[:], ntn2[:], -1.0, 2.0, AO.mult, AO.add)
                nc.vector.tensor_tensor(ar0[:], ar0[:], ntn2[:], AO.mult)
                al_nat = smp.tile([128, NTC], dt.float32, tag="al_nat")
                nc.vector.tensor_tensor(al_nat[:], rr[:], ar0[:], AO.mult)
                nc.vector.tensor_scalar(al_nat[:], al_nat[:], 127.0, None, AO.mult)
                nc.sync.dma_start(
                    row_bounce[g, 3].rearrange("(c p) -> p c", p=128)[:], al_nat[:])
                al_row = rowp.tile([1, TG], dt.float32, tag="grow")
                nc.sync.dma_start(al_row[:],
                                  row_bounce[g, 3].rearrange("(o f) -> o f", o=1)[:])
                alt_ps = ps_misc.tile([128, TG], dt.float32, tag="misc_ps")
                nc.tensor.matmul(alt_ps[:], ones_row[:], al_row[:], start=True,
                                 stop=True)
                al_tile = bcp.tile([128, TG], dt.float32, tag="al_tile")
                nc.scalar.copy(al_tile[:], alt_ps[:])

                # ---- phase 2b: quantize h ----
                qhT = qhp.tile([128, IC * TG], dt.bfloat16, tag="qhT")
                for ic in range(IC):
                    tq = evp.tile([128, TG], dt.float32, tag="hq_t")
                    nc.vector.scalar_tensor_tensor(tq[:], hT[:, ic * TG:(ic + 1) * TG],
                                                   lnw_sb[:, ic:ic + 1], al_tile[:],
                                                   AO.mult, AO.mult)
                    nc.vector.tensor_scalar(tq[:], tq[:], MAGIC, -MAGIC, AO.add,
                                            AO.add)
                    nc.vector.tensor_scalar(qhT[:, ic * TG:(ic + 1) * TG], tq[:],
                                            127.0, -128.0, AO.min, AO.max)

                # ---- phase 2c: down matmuls + dequant + store partial ----
                for tcx in range(NTC):
                    for nh in range(NH):
                        y_ps = ps_dn.tile([128, 512], dt.float32, tag="y_ps")
                        for ic in range(IC):
                            nc.tensor.matmul(
                                y_ps[:],
                                qhT[:, ic * TG + tcx * 128: ic * TG + (tcx + 1) * 128],
                                qwd[:, ic * 2048 + nh * 512: ic * 2048 + (nh + 1) * 512],
                                start=(ic == 0), stop=(ic == IC - 1))
                        y_sb = evp.tile([128, 512], dt.float32, tag="y_sb")
                        nc.vector.tensor_scalar(y_sb[:], y_ps[:],
                                                cd[:, tcx:tcx + 1], None, AO.mult)
                        nc.sync.dma_start(
                            y_partial[tok0 + tcx * 128: tok0 + (tcx + 1) * 128,
                                      nh * 512:(nh + 1) * 512], y_sb[:])

                # ---- reduce-scatter every RS_BATCH groups ----
                if g % RS_BATCH == RS_BATCH - 1:
                    b = g // RS_BATCH
                    rows0 = b * RS_BATCH * TG
                    nrows = RS_BATCH * TG
                    rs_out = dram_rs.tile([nrows // N_CORES, 2048], dt.float32,
                                          tag="rs_out")
                    nc.gpsimd.collective_compute(
                        "ReduceScatter", AO.add, replica_groups=RG,
                        ins=[y_partial[rows0:rows0 + nrows, :].opt()],
                        outs=[rs_out.opt()])
                    nc.sync.dma_start(
                        y_out[b * (nrows // N_CORES):(b + 1) * (nrows // N_CORES), :],
                        rs_out[:])

    nc.compile()
    return nc


def _get_nc():
    if "nc" not in _CACHED:
        _CACHED["nc"] = _build()
    return _CACHED["nc"]


def kernel(x, w_gate, w_up, w_down, ln_weight):
    from concourse import bass_utils

    nc = _get_nc()

    xf = np.ascontiguousarray(x.reshape(T, H).T, dtype=np.float32)
    wgT = np.ascontiguousarray(w_gate.T, dtype=np.float32)   # [H, I]
    wuT = np.ascontiguousarray(w_up.T, dtype=np.float32)     # [H, I]
    wdT = np.ascontiguousarray(w_down.T, dtype=np.float32)   # [I, H]
    lnw = np.asarray(ln_weight, dtype=np.float32)

    in_maps = []
    for r in range(N_CORES):
        c0 = r * ISH
        in_maps.append({
            "xT": xf,
            "wgT": np.ascontiguousarray(wgT[:, c0:c0 + ISH]),
            "wuT": np.ascontiguousarray(wuT[:, c0:c0 + ISH]),
            "wdT": np.ascontiguousarray(wdT[c0:c0 + ISH, :]),
            "lnw": np.ascontiguousarray(lnw[c0:c0 + ISH]),
        })

    res = bass_utils.run_bass_kernel_spmd(nc, in_maps,
                                          core_ids=list(range(N_CORES)))

    # reassemble: RS batch b gave core r tokens [1024*b + 128*r, +128)
    out = np.empty((T, 2048), dtype=np.float32)
    rows_per_batch = RS_BATCH * TG // N_CORES          # 128
    for r in range(N_CORES):
        yr = res.results[r]["y_out"]
        for b in range(NB):
            t0 = b * RS_BATCH * TG + r * rows_per_batch
            out[t0:t0 + rows_per_batch] = yr[b * rows_per_batch:(b + 1) * rows_per_batch]
    return out.reshape(B, S, 2048)


# revision 13
# speedup vs baseline: 1.0130x; 1.0130x over previous
"""BitnetMLP on 8 TRN2 NeuronCores — Megatron tensor-parallel over the
intermediate dim I, with exact integer arithmetic on the TensorEngine.

Math: activation fake-quant makes activations exact int8 values and weight
fake-quant makes weights exact ternary values. Both are exactly representable
in bf16/fp8e4, and PSUM accumulates in f32, so every matmul is computed as an
exact integer matmul at full bf16 speed; per-token / per-tensor dequant scales
are applied to the f32 partial sums afterward.

Sharding (per core r of 8):
  w_gate/w_up: I-column shard (1024 of 8192)  -> h^T shard [I_sh=1024, T]
  w_down:      I-row shard                    -> partial y, ReduceScatter(add)
  per-token RMS var and abs-max stats over the full I: AllReduce add / max.

Layouts are feature-major (host pre-transposes x and the weights so the
contract dim lands on SBUF partitions; no on-device transposes of x/w/h).

Structure: an x-quant prepass streams exact-int bf16 x^T tiles to DRAM so the
main per-group matmul pipeline has no latency chains (DRAM gathers / AllReduce
waits overlap matmuls of neighboring groups).
"""
import numpy as np

N_CORES = 8
B, S, H, I = 2, 2048, 2048, 8192
T = B * S                      # 4096 tokens
ISH = I // N_CORES             # 1024  I shard per core
TG = 512                       # tokens per group
NG = T // TG                   # 8 groups
KC = H // 128                  # 16 contract chunks for gate/up
IC = ISH // 128                # 8  contract chunks for down / h^T partition chunks
NH = 2048 // 512               # 4  output col groups for down
NTC = TG // 128                # 4  token tiles per group
RS_BATCH = 2                   # groups per ReduceScatter
NB = NG // RS_BATCH            # 4 RS batches

MAGIC = float(1.5 * 2 ** 23)   # f32 round-to-nearest-even forcing constant
EPS = 1e-5
RMS_EPS = 1e-6

_CACHED = {}


def _build():
    import concourse.bass as bass
    import concourse.bacc as bacc
    import concourse.tile as tile
    import concourse.mybir as mybir
    from concourse import masks
    from contextlib import ExitStack

    dt = mybir.dt
    AO = mybir.AluOpType
    AF = mybir.ActivationFunctionType
    RG = [list(range(N_CORES))]

    nc = bacc.Bacc("TRN2", target_bir_lowering=False, debug=False,
                   num_devices=N_CORES)

    xT_in = nc.dram_tensor("xT", [H, T], dt.float32, kind="ExternalInput")
    wgT_in = nc.dram_tensor("wgT", [H, ISH], dt.float32, kind="ExternalInput")
    wuT_in = nc.dram_tensor("wuT", [H, ISH], dt.float32, kind="ExternalInput")
    wdT_in = nc.dram_tensor("wdT", [ISH, 2048], dt.float32, kind="ExternalInput")
    lnw_in = nc.dram_tensor("lnw", [ISH], dt.float32, kind="ExternalInput")
    y_out = nc.dram_tensor("y_out", [T // N_CORES, 2048], dt.float32,
                           kind="ExternalOutput")

    with tile.TileContext(nc) as tc:
        with ExitStack() as stack:
            ep = stack.enter_context
            constp = ep(tc.tile_pool(name="const", bufs=1))
            wqp = ep(tc.tile_pool(name="wq", bufs=1))
            wstage = ep(tc.tile_pool(name="wstage", bufs=2))
            xstage = ep(tc.tile_pool(name="xstage", bufs=2))
            qxp = ep(tc.tile_pool(name="qx", bufs=2))
            hbp = ep(tc.tile_pool(name="hbuf", bufs=2))
            qhp = ep(tc.tile_pool(name="qh", bufs=2))
            bcp = ep(tc.tile_pool(name="bc", bufs=2))
            smp = ep(tc.tile_pool(name="small", bufs=2))
            rowp = ep(tc.tile_pool(name="rows", bufs=2))
            rowp2 = ep(tc.tile_pool(name="rows2", bufs=1))
            evp = ep(tc.tile_pool(name="evac", bufs=2))
            ps_gu = ep(tc.tile_pool(name="ps_gu", bufs=2, space="PSUM"))
            ps_dn = ep(tc.tile_pool(name="ps_dn", bufs=2, space="PSUM"))
            ps_ss = ep(tc.tile_pool(name="ps_ss", bufs=1, space="PSUM"))
            ps_misc = ep(tc.tile_pool(name="ps_misc", bufs=1, space="PSUM"))
            dram = ep(tc.tile_pool(name="dram", bufs=1, space="DRAM"))
            dram_rs = ep(tc.tile_pool(name="dram_rs", bufs=2, space="DRAM"))

            # ---------- constants ----------
            ident = constp.tile([128, 128], dt.float32)
            masks.make_identity(nc, ident[:])
            ones_col = constp.tile([128, 1], dt.float32)   # lhsT for partition sums
            nc.vector.memset(ones_col[:], 1.0)
            ones_row = constp.tile([1, 128], dt.float32)   # lhsT for K=1 broadcasts
            nc.vector.memset(ones_row[:], 1.0)
            lnw_sb = constp.tile([128, IC], dt.float32)    # lnw[128*ic + p] at [p, ic]
            nc.sync.dma_start(lnw_sb[:], lnw_in.rearrange("(c p) -> p c", p=128)[:])
            alnw_sb = constp.tile([128, IC], dt.float32)   # |lnw|
            nc.vector.tensor_scalar(alnw_sb.bitcast(dt.uint32)[:],
                                    lnw_sb.bitcast(dt.uint32)[:],
                                    0x7FFFFFFF, None, AO.bitwise_and)

            # ---------- internal DRAM ----------
            y_partial = dram.tile([T, 2048], dt.float32)
            qx_dram = dram.tile([NG, KC, 128, TG], dt.bfloat16)
            ss_part = dram.tile([T], dt.float32)
            ss_glob = dram.tile([T], dt.float32)
            pm_part = dram.tile([T], dt.float32)
            pm_glob = dram.tile([T], dt.float32)
            wsum_part = dram.tile([8], dt.float32)
            wsum_glob = dram.tile([8], dt.float32)
            row_bounce = dram.tile([NG, 4, TG], dt.float32)  # sx / cg+cu / al / spare

            # ---------- weight abs-sum stats ----------
            wsum_row = rowp.tile([1, 8], dt.float32, tag="wsum_row")
            for wi, (w_in, nchunk, wcols) in enumerate((
                    (wgT_in, KC, ISH), (wuT_in, KC, ISH), (wdT_in, IC, 2048))):
                acc = smp.tile([128, 1], dt.float32, tag="wacc")
                for c in range(nchunk):
                    for cc in range(wcols // 512):
                        st = wstage.tile([128, 512], dt.float32, tag="wstage")
                        nc.sync.dma_start(st[:], w_in[c * 128:(c + 1) * 128,
                                                      cc * 512:(cc + 1) * 512])
                        red = smp.tile([128, 1], dt.float32, tag="wred")
                        nc.vector.tensor_reduce(red[:], st[:], mybir.AxisListType.X,
                                                AO.add, apply_absolute_value=True)
                        if c == 0 and cc == 0:
                            nc.vector.tensor_copy(acc[:], red[:])
                        else:
                            nc.vector.tensor_tensor(acc[:], acc[:], red[:], AO.add)
                wsum_ps = ps_misc.tile([128, 512], dt.float32, tag="misc_ps")
                nc.tensor.matmul(wsum_ps[0:1, 0:1], ones_col[:], acc[:], start=True,
                                 stop=True)
                nc.scalar.copy(wsum_row[:, wi:wi + 1], wsum_ps[0:1, 0:1])
            nc.vector.memset(wsum_row[:, 3:8], 0.0)
            nc.sync.dma_start(wsum_part.rearrange("(o f) -> o f", o=1)[:], wsum_row[:])
            nc.gpsimd.collective_compute(
                "AllReduce", AO.add, replica_groups=RG,
                ins=[wsum_part.opt()], outs=[wsum_glob.opt()])

            # scl_row: [sw_g, sw_u, sw_d, mg/127, mu/127, md, 0, 0]
            wsg_row = rowp.tile([1, 8], dt.float32, tag="wsg_row")
            nc.sync.dma_start(wsg_row[:], wsum_glob.rearrange("(o f) -> o f", o=1)[:])
            mean_row = rowp.tile([1, 8], dt.float32, tag="mean_row")
            nc.vector.tensor_scalar(mean_row[:, 0:3], wsg_row[:, 0:3],
                                    float(1.0 / (I * H)), EPS, AO.mult, AO.max)
            scl_row = rowp.tile([1, 8], dt.float32, tag="scl_row")
            rw = rowp.tile([1, 8], dt.float32, tag="rw_row")
            nc.vector.reciprocal(rw[:, 0:3], mean_row[:, 0:3])
            nt = rowp.tile([1, 8], dt.float32, tag="nt_row")
            nc.vector.tensor_tensor(nt[:, 0:3], mean_row[:, 0:3], rw[:, 0:3], AO.mult)
            nc.vector.tensor_scalar(nt[:, 0:3], nt[:, 0:3], -1.0, 2.0, AO.mult, AO.add)
            nc.vector.tensor_tensor(scl_row[:, 0:3], rw[:, 0:3], nt[:, 0:3], AO.mult)
            nc.vector.tensor_scalar(scl_row[:, 3:5], mean_row[:, 0:2],
                                    float(1.0 / 127.0), None, AO.mult)
            nc.vector.tensor_copy(scl_row[:, 5:6], mean_row[:, 2:3])
            nc.vector.memset(scl_row[:, 6:8], 0.0)
            wst_ps = ps_misc.tile([128, 512], dt.float32, tag="misc_ps")
            nc.tensor.matmul(wst_ps[:, 0:8], ones_row[:], scl_row[:], start=True,
                             stop=True)
            wstats = constp.tile([128, 8], dt.float32)
            nc.vector.tensor_copy(wstats[:], wst_ps[:, 0:8])

            # ---------- quantize weights to ternary fp8 ----------
            qwg = wqp.tile([128, KC * ISH], dt.float8e4)
            qwu = wqp.tile([128, KC * ISH], dt.float8e4)
            qwd = wqp.tile([128, IC * 2048], dt.float8e4)
            for (w_in, qw, nchunk, wcols, si) in (
                (wgT_in, qwg, KC, ISH, 0), (wuT_in, qwu, KC, ISH, 1),
                (wdT_in, qwd, IC, 2048, 2),
            ):
                for c in range(nchunk):
                    for cc in range(wcols // 512):
                        st = wstage.tile([128, 512], dt.float32, tag="wstage")
                        nc.sync.dma_start(st[:], w_in[c * 128:(c + 1) * 128,
                                                      cc * 512:(cc + 1) * 512])
                        t1 = wstage.tile([128, 512], dt.float32, tag="wq_t1")
                        nc.vector.tensor_scalar(t1[:], st[:], wstats[:, si:si + 1],
                                                MAGIC, AO.mult, AO.add)
                        nc.vector.tensor_scalar(t1[:], t1[:], -MAGIC, 1.0, AO.add,
                                                AO.min)
                        o0 = c * wcols + cc * 512
                        nc.vector.tensor_scalar(qw[:, o0:o0 + 512], t1[:],
                                                -1.0, None, AO.max)

            # ---------- x-quant prepass: stream exact-int bf16 x^T to DRAM ----
            for g in range(NG):
                tok0 = g * TG
                xmax = smp.tile([128, TG], dt.float32, tag="xmax")
                for kc in range(KC):
                    st = xstage.tile([128, TG], dt.float32, tag="xs1")
                    nc.sync.dma_start(st[:], xT_in[kc * 128:(kc + 1) * 128,
                                                   tok0:tok0 + TG])
                    if kc == 0:
                        nc.vector.tensor_scalar(xmax.bitcast(dt.uint32)[:],
                                                st.bitcast(dt.uint32)[:],
                                                0x7FFFFFFF, None, AO.bitwise_and)
                    else:
                        nc.vector.tensor_scalar(st.bitcast(dt.uint32)[:],
                                                st.bitcast(dt.uint32)[:],
                                                0x7FFFFFFF, None, AO.bitwise_and)
                        nc.vector.tensor_tensor(xmax[:], xmax[:], st[:], AO.max)
                mx_nat = smp.tile([128, NTC], dt.float32, tag="mx_nat")
                for c in range(NTC):
                    tr_ps = ps_misc.tile([128, 512], dt.float32, tag="misc_ps")
                    nc.tensor.transpose(tr_ps[:, 0:128],
                                        xmax[:, c * 128:(c + 1) * 128], ident[:])
                    nc.vector.tensor_reduce(mx_nat[:, c:c + 1], tr_ps[:, 0:128],
                                            mybir.AxisListType.X, AO.max)
                nc.vector.tensor_scalar(mx_nat[:], mx_nat[:], EPS, None, AO.max)
                # sx = 127/mxc (reciprocal + newton)
                r0 = smp.tile([128, NTC], dt.float32, tag="sx_r0")
                nc.vector.reciprocal(r0[:], mx_nat[:])
                ntr = smp.tile([128, NTC], dt.float32, tag="sx_nt")
                nc.vector.tensor_tensor(ntr[:], mx_nat[:], r0[:], AO.mult)
                nc.vector.tensor_scalar(ntr[:], ntr[:], -1.0, 2.0, AO.mult, AO.add)
                sxn = smp.tile([128, NTC], dt.float32, tag="sxn")
                nc.vector.tensor_tensor(sxn[:], r0[:], ntr[:], AO.mult)
                nc.vector.tensor_scalar(sxn[:], sxn[:], 127.0, None, AO.mult)
                # cg/cu rows for the main loop, packed as [p, (s c)] s=0:cg 1:cu
                cgcu = smp.tile([128, 2 * NTC], dt.float32, tag="cgcu")
                nc.vector.tensor_scalar(cgcu[:, 0:NTC], mx_nat[:], wstats[:, 3:4],
                                        None, AO.mult)
                nc.vector.tensor_scalar(cgcu[:, NTC:2 * NTC], mx_nat[:],
                                        wstats[:, 4:5], None, AO.mult)
                nc.sync.dma_start(
                    row_bounce[g, 1:3].rearrange("s (c p) -> p s c", p=128)[:],
                    cgcu.rearrange("p (s c) -> p s c", c=NTC)[:])
                # sx broadcast (local to the prepass)
                nc.sync.dma_start(
                    row_bounce[g, 0].rearrange("(c p) -> p c", p=128)[:], sxn[:])
                sx_row = rowp.tile([1, TG], dt.float32, tag="grow")
                nc.sync.dma_start(
                    sx_row[:], row_bounce[g, 0].rearrange("(o f) -> o f", o=1)[:])
                sx_ps = ps_misc.tile([128, 512], dt.float32, tag="misc_ps")
                nc.tensor.matmul(sx_ps[:, 0:TG], ones_row[:], sx_row[:], start=True,
                                 stop=True)
                sx_tile = bcp.tile([128, TG], dt.float32, tag="sxal_tile")
                nc.scalar.copy(sx_tile[:], sx_ps[:, 0:TG])
                for kc in range(KC):
                    st = xstage.tile([128, TG], dt.float32, tag="xs2")
                    nc.sync.dma_start(st[:], xT_in[kc * 128:(kc + 1) * 128,
                                                   tok0:tok0 + TG])
                    tq = xstage.tile([128, TG], dt.float32, tag="xq_t")
                    nc.vector.tensor_tensor(tq[:], st[:], sx_tile[:], AO.mult)
                    nc.vector.tensor_scalar(tq[:], tq[:], MAGIC, -MAGIC, AO.add,
                                            AO.add)
                    qx_sb = xstage.tile([128, TG], dt.bfloat16, tag="qx_sb")
                    nc.vector.tensor_scalar(qx_sb[:], tq[:], 127.0, -128.0, AO.min,
                                            AO.max)
                    nc.sync.dma_start(qx_dram[g, kc], qx_sb[:])

            # ---------- main pipeline over token groups ----------
            for g in range(NG):
                tok0 = g * TG
                # load quantized x group [128, KC*TG]
                qxT = qxp.tile([128, KC * TG], dt.bfloat16, tag="qxT")
                for kc in range(KC):
                    nc.sync.dma_start(qxT[:, kc * TG:(kc + 1) * TG], qx_dram[g, kc])
                # cg/cu broadcast tiles
                ccrow = rowp2.tile([1, 2 * TG], dt.float32, tag="grow2")
                nc.sync.dma_start(
                    ccrow[:], row_bounce[g, 1:3]
                    .rearrange("s f -> (s f)").rearrange("(o f) -> o f", o=1)[:])
                cg_tile = bcp.tile([128, TG], dt.float32, tag="cg_tile")
                cu_tile = bcp.tile([128, TG], dt.float32, tag="cu_tile")
                for ri, tile_ in ((0, cg_tile), (1, cu_tile)):
                    bc_ps = ps_misc.tile([128, 512], dt.float32, tag="misc_ps")
                    nc.tensor.matmul(bc_ps[:, 0:TG], ones_row[:],
                                     ccrow[:, ri * TG:(ri + 1) * TG],
                                     start=True, stop=True)
                    nc.scalar.copy(tile_[:], bc_ps[:, 0:TG])

                # gate/up matmuls + h + stats
                hT = hbp.tile([128, IC * TG], dt.float32, tag="hT")
                maxt = smp.tile([128, TG], dt.float32, tag="maxt")
                nc.vector.memset(maxt[:], 0.0)
                ss_ps = ps_ss.tile([1, TG], dt.float32, tag="ss_ps")
                for ic in range(IC):
                    g_ps = ps_gu.tile([128, TG], dt.float32, tag="g_ps")
                    u_ps = ps_gu.tile([128, TG], dt.float32, tag="u_ps")
                    for kc in range(KC):
                        nc.tensor.matmul(
                            g_ps[:],
                            qwg[:, kc * ISH + ic * 128: kc * ISH + (ic + 1) * 128],
                            qxT[:, kc * TG:(kc + 1) * TG],
                            start=(kc == 0), stop=(kc == KC - 1))
                    for kc in range(KC):
                        nc.tensor.matmul(
                            u_ps[:],
                            qwu[:, kc * ISH + ic * 128: kc * ISH + (ic + 1) * 128],
                            qxT[:, kc * TG:(kc + 1) * TG],
                            start=(kc == 0), stop=(kc == KC - 1))
                    gv = evp.tile([128, TG], dt.float32, tag="gv")
                    nc.vector.tensor_tensor(gv[:], g_ps[:], cg_tile[:], AO.mult)
                    sv = evp.tile([128, TG], dt.float32, tag="sv")
                    nc.scalar.activation(sv[:], gv[:], AF.Silu)
                    uv = evp.tile([128, TG], dt.float32, tag="uv")
                    nc.vector.tensor_tensor(uv[:], u_ps[:], cu_tile[:], AO.mult)
                    hslice = hT[:, ic * TG:(ic + 1) * TG]
                    nc.vector.tensor_tensor(hslice, sv[:], uv[:], AO.mult)
                    h2 = evp.tile([128, TG], dt.float32, tag="h2")
                    nc.scalar.square(h2[:], hslice)
                    nc.tensor.matmul(ss_ps[:], ones_col[:], h2[:],
                                     start=(ic == 0), stop=(ic == IC - 1))
                    ha = evp.tile([128, TG], dt.float32, tag="h2")
                    nc.scalar.activation(ha[:], hslice, AF.Abs)
                    nc.vector.scalar_tensor_tensor(maxt[:], ha[:],
                                                   alnw_sb[:, ic:ic + 1], maxt[:],
                                                   AO.mult, AO.max)
                pm_nat = smp.tile([128, NTC], dt.float32, tag="pm_nat")
                for c in range(NTC):
                    tr_ps = ps_misc.tile([128, 512], dt.float32, tag="misc_ps")
                    nc.tensor.transpose(tr_ps[:, 0:128],
                                        maxt[:, c * 128:(c + 1) * 128], ident[:])
                    nc.vector.tensor_reduce(pm_nat[:, c:c + 1], tr_ps[:, 0:128],
                                            mybir.AxisListType.X, AO.max)
                ss_row = rowp.tile([1, TG], dt.float32, tag="grow")
                nc.vector.tensor_copy(ss_row[:], ss_ps[:])
                nc.sync.dma_start(ss_part[tok0:tok0 + TG]
                                  .rearrange("(o f) -> o f", o=1)[:], ss_row[:])
                nc.sync.dma_start(pm_part[tok0:tok0 + TG]
                                  .rearrange("(c p) -> p c", p=128)[:], pm_nat[:])
                nc.gpsimd.collective_compute(
                    "AllReduce", AO.add, replica_groups=RG,
                    ins=[ss_part[tok0:tok0 + TG].opt()],
                    outs=[ss_glob[tok0:tok0 + TG].opt()])
                nc.gpsimd.collective_compute(
                    "AllReduce", AO.max, replica_groups=RG,
                    ins=[pm_part[tok0:tok0 + TG].opt()],
                    outs=[pm_glob[tok0:tok0 + TG].opt()])

                # global per-token scales
                ssg = smp.tile([128, NTC], dt.float32, tag="ssg")
                nc.sync.dma_start(ssg[:], ss_glob[tok0:tok0 + TG]
                                  .rearrange("(c p) -> p c", p=128)[:])
                pmg = smp.tile([128, NTC], dt.float32, tag="pmg")
                nc.sync.dma_start(pmg[:], pm_glob[tok0:tok0 + TG]
                                  .rearrange("(c p) -> p c", p=128)[:])
                vr = smp.tile([128, NTC], dt.float32, tag="vr")
                nc.vector.tensor_scalar(vr[:], ssg[:], float(1.0 / I), RMS_EPS,
                                        AO.mult, AO.add)
                sq = smp.tile([128, NTC], dt.float32, tag="sq")
                nc.scalar.sqrt(sq[:], vr[:])
                rr = smp.tile([128, NTC], dt.float32, tag="rr")
                nc.vector.reciprocal(rr[:], sq[:])
                ntn = smp.tile([128, NTC], dt.float32, tag="ntn")
                nc.vector.tensor_tensor(ntn[:], sq[:], rr[:], AO.mult)
                nc.vector.tensor_scalar(ntn[:], ntn[:], -1.0, 2.0, AO.mult, AO.add)
                nc.vector.tensor_tensor(rr[:], rr[:], ntn[:], AO.mult)
                rmc = smp.tile([128, NTC], dt.float32, tag="rmc")
                nc.vector.tensor_tensor(rmc[:], rr[:], pmg[:], AO.mult)
                nc.vector.tensor_scalar(rmc[:], rmc[:], EPS, None, AO.max)
                cd = smp.tile([128, NTC], dt.float32, tag="cd")
                nc.vector.tensor_scalar(cd[:], rmc[:], wstats[:, 5:6],
                                        float(1.0 / 127.0), AO.mult, AO.mult)
                ar0 = smp.tile([128, NTC], dt.float32, tag="ar0")
                nc.vector.reciprocal(ar0[:], rmc[:])
                ntn2 = smp.tile([128, NTC], dt.float32, tag="ntn2")
                nc.vector.tensor_tensor(ntn2[:], rmc[:], ar0[:], AO.mult)
                nc.vector.tensor_scalar(ntn2[:], ntn2[:], -1.0, 2.0, AO.mult, AO.add)
                nc.vector.tensor_tensor(ar0[:], ar0[:], ntn2[:], AO.mult)
                al_nat = smp.tile([128, NTC], dt.float32, tag="al_nat")
                nc.vector.tensor_tensor(al_nat[:], rr[:], ar0[:], AO.mult)
                nc.vector.tensor_scalar(al_nat[:], al_nat[:], 127.0, None, AO.mult)
                nc.sync.dma_start(
                    row_bounce[g, 3].rearrange("(c p) -> p c", p=128)[:], al_nat[:])
                al_row = rowp.tile([1, TG], dt.float32, tag="grow")
                nc.sync.dma_start(al_row[:],
                                  row_bounce[g, 3].rearrange("(o f) -> o f", o=1)[:])
                alt_ps = ps_misc.tile([128, 512], dt.float32, tag="misc_ps")
                nc.tensor.matmul(alt_ps[:, 0:TG], ones_row[:], al_row[:], start=True,
                                 stop=True)
                al_tile = bcp.tile([128, TG], dt.float32, tag="sxal_tile")
                nc.scalar.copy(al_tile[:], alt_ps[:, 0:TG])

                # quantize h
                qhT = qhp.tile([128, IC * TG], dt.bfloat16, tag="qhT")
                for ic in range(IC):
                    tq = evp.tile([128, TG], dt.float32, tag="hq_t")
                    nc.vector.scalar_tensor_tensor(tq[:], hT[:, ic * TG:(ic + 1) * TG],
                                                   lnw_sb[:, ic:ic + 1], al_tile[:],
                                                   AO.mult, AO.mult)
                    nc.vector.tensor_scalar(tq[:], tq[:], MAGIC, -MAGIC, AO.add,
                                            AO.add)
                    nc.vector.tensor_scalar(qhT[:, ic * TG:(ic + 1) * TG], tq[:],
                                            127.0, -128.0, AO.min, AO.max)

                # down matmuls + dequant + store partial
                for tcx in range(NTC):
                    for nh in range(NH):
                        y_ps = ps_dn.tile([128, 512], dt.float32, tag="y_ps")
                        for ic in range(IC):
                            nc.tensor.matmul(
                                y_ps[:],
                                qhT[:, ic * TG + tcx * 128: ic * TG + (tcx + 1) * 128],
                                qwd[:, ic * 2048 + nh * 512: ic * 2048 + (nh + 1) * 512],
                                start=(ic == 0), stop=(ic == IC - 1))
                        y_sb = evp.tile([128, 512], dt.float32, tag="y_sb")
                        nc.scalar.mul(y_sb[:], y_ps[:], cd[:, tcx:tcx + 1])
                        nc.sync.dma_start(
                            y_partial[tok0 + tcx * 128: tok0 + (tcx + 1) * 128,
                                      nh * 512:(nh + 1) * 512], y_sb[:])

                # reduce-scatter every RS_BATCH groups
                if g % RS_BATCH == RS_BATCH - 1:
                    b = g // RS_BATCH
                    rows0 = b * RS_BATCH * TG
                    nrows = RS_BATCH * TG
                    rs_out = dram_rs.tile([nrows // N_CORES, 2048], dt.float32,
                                          tag="rs_out")
                    nc.gpsimd.collective_compute(
                        "ReduceScatter", AO.add, replica_groups=RG,
                        ins=[y_partial[rows0:rows0 + nrows, :].opt()],
                        outs=[rs_out.opt()])
                    nc.sync.dma_start(
                        y_out[b * (nrows // N_CORES):(b + 1) * (nrows // N_CORES), :],
                        rs_out[:])

    nc.compile()
    return nc


def _get_nc():
    if "nc" not in _CACHED:
        _CACHED["nc"] = _build()
    return _CACHED["nc"]


def _make_in_maps(x, w_gate, w_up, w_down, ln_weight):
    xf = np.ascontiguousarray(np.asarray(x, dtype=np.float32).reshape(T, H).T)
    wgT = np.asarray(w_gate, dtype=np.float32).T   # [H, I]
    wuT = np.asarray(w_up, dtype=np.float32).T     # [H, I]
    wdT = np.asarray(w_down, dtype=np.float32).T   # [I, H]
    lnw = np.asarray(ln_weight, dtype=np.float32)
    in_maps = []
    for r in range(N_CORES):
        c0 = r * ISH
        in_maps.append({
            "xT": xf,
            "wgT": np.ascontiguousarray(wgT[:, c0:c0 + ISH]),
            "wuT": np.ascontiguousarray(wuT[:, c0:c0 + ISH]),
            "wdT": np.ascontiguousarray(wdT[c0:c0 + ISH, :]),
            "lnw": np.ascontiguousarray(lnw[c0:c0 + ISH]),
        })
    return in_maps


def _assemble(results):
    out = np.empty((T, 2048), dtype=np.float32)
    rows_per_batch = RS_BATCH * TG // N_CORES          # 128
    for r in range(N_CORES):
        yr = results[r]["y_out"]
        for b in range(NB):
            t0 = b * RS_BATCH * TG + r * rows_per_batch
            out[t0:t0 + rows_per_batch] = \
                yr[b * rows_per_batch:(b + 1) * rows_per_batch]
    return out.reshape(B, S, 2048)


def kernel(x, w_gate, w_up, w_down, ln_weight):
    from concourse import bass_utils

    nc = _get_nc()
    in_maps = _make_in_maps(x, w_gate, w_up, w_down, ln_weight)
    res = bass_utils.run_bass_kernel_spmd(nc, in_maps,
                                          core_ids=list(range(N_CORES)))
    return _assemble(res.results)


# revision 15
# speedup vs baseline: 1.2338x; 1.2180x over previous
"""BitnetMLP on 8 TRN2 NeuronCores — Megatron tensor-parallel over the
intermediate dim I, with exact integer arithmetic on the TensorEngine.

Math: activation fake-quant makes activations exact int8 values and weight
fake-quant makes weights exact ternary values. Both are exactly representable
in bf16/fp8e4, and PSUM accumulates in f32, so every matmul is computed as an
exact integer matmul at full bf16 speed; per-token / per-tensor dequant scales
are applied to the f32 partial sums afterward.

Sharding (per core r of 8):
  w_gate/w_up: I-column shard (1024 of 8192)  -> h^T shard [I_sh=1024, T]
  w_down:      I-row shard                    -> partial y, ReduceScatter(add)
  per-token RMS var and abs-max stats over the full I: AllReduce add / max.

Layouts are feature-major (host pre-transposes x and the weights so the
contract dim lands on SBUF partitions; no on-device transposes of x/w/h).

Structure: an x-quant prepass streams exact-int bf16 x^T tiles to DRAM so the
main per-group matmul pipeline has no latency chains (DRAM gathers / AllReduce
waits overlap matmuls of neighboring groups).
"""
import numpy as np

N_CORES = 8
B, S, H, I = 2, 2048, 2048, 8192
T = B * S                      # 4096 tokens
ISH = I // N_CORES             # 1024  I shard per core
TG = 512                       # tokens per group
NG = T // TG                   # 8 groups
KC = H // 128                  # 16 contract chunks for gate/up
IC = ISH // 128                # 8  contract chunks for down / h^T partition chunks
NH = 2048 // 512               # 4  output col groups for down
NTC = TG // 128                # 4  token tiles per group
RS_BATCH = 2                   # groups per ReduceScatter
NB = NG // RS_BATCH            # 4 RS batches

MAGIC = float(1.5 * 2 ** 23)   # f32 round-to-nearest-even forcing constant
EPS = 1e-5
RMS_EPS = 1e-6

_CACHED = {}


def _build():
    import concourse.bass as bass
    import concourse.bacc as bacc
    import concourse.tile as tile
    import concourse.mybir as mybir
    from concourse import masks
    from contextlib import ExitStack

    dt = mybir.dt
    AO = mybir.AluOpType
    AF = mybir.ActivationFunctionType
    RG = [list(range(N_CORES))]

    nc = bacc.Bacc("TRN2", target_bir_lowering=False, debug=False,
                   num_devices=N_CORES)

    xT_in = nc.dram_tensor("xT", [H, T], dt.float32, kind="ExternalInput")
    wgT_in = nc.dram_tensor("wgT", [H, ISH], dt.float32, kind="ExternalInput")
    wuT_in = nc.dram_tensor("wuT", [H, ISH], dt.float32, kind="ExternalInput")
    wdT_in = nc.dram_tensor("wdT", [ISH, 2048], dt.float32, kind="ExternalInput")
    lnw_in = nc.dram_tensor("lnw", [ISH], dt.float32, kind="ExternalInput")
    y_out = nc.dram_tensor("y_out", [T // N_CORES, 2048], dt.float32,
                           kind="ExternalOutput")

    with tile.TileContext(nc) as tc:
        with ExitStack() as stack:
            ep = stack.enter_context
            constp = ep(tc.tile_pool(name="const", bufs=1))
            wqp = ep(tc.tile_pool(name="wq", bufs=1))
            wstage = ep(tc.tile_pool(name="wstage", bufs=2))
            xstage = ep(tc.tile_pool(name="xstage", bufs=2))
            qxp = ep(tc.tile_pool(name="qx", bufs=2))
            hbp = ep(tc.tile_pool(name="hbuf", bufs=2))
            qhp = ep(tc.tile_pool(name="qh", bufs=2))
            bcp = ep(tc.tile_pool(name="bc", bufs=2))
            sxp = ep(tc.tile_pool(name="sxal", bufs=1))
            yrp = ep(tc.tile_pool(name="yrow", bufs=1))
            smp = ep(tc.tile_pool(name="small", bufs=2))
            rowp = ep(tc.tile_pool(name="rows", bufs=2))
            rowp2 = ep(tc.tile_pool(name="rows2", bufs=1))
            evp = ep(tc.tile_pool(name="evac", bufs=2))
            ps_gu = ep(tc.tile_pool(name="ps_gu", bufs=2, space="PSUM"))
            ps_dn = ep(tc.tile_pool(name="ps_dn", bufs=2, space="PSUM"))
            ps_ss = ep(tc.tile_pool(name="ps_ss", bufs=1, space="PSUM"))
            ps_misc = ep(tc.tile_pool(name="ps_misc", bufs=1, space="PSUM"))
            dram = ep(tc.tile_pool(name="dram", bufs=1, space="DRAM"))
            dram_rs = ep(tc.tile_pool(name="dram_rs", bufs=2, space="DRAM"))

            # ---------- constants ----------
            ident = constp.tile([128, 128], dt.float32)
            masks.make_identity(nc, ident[:])
            ones_col = constp.tile([128, 1], dt.float32)   # lhsT for partition sums
            nc.vector.memset(ones_col[:], 1.0)
            ones_row = constp.tile([1, 128], dt.float32)   # lhsT for K=1 broadcasts
            nc.vector.memset(ones_row[:], 1.0)
            lnw_sb = constp.tile([128, IC], dt.float32)    # lnw[128*ic + p] at [p, ic]
            nc.sync.dma_start(lnw_sb[:], lnw_in.rearrange("(c p) -> p c", p=128)[:])
            alnw_sb = constp.tile([128, IC], dt.float32)   # |lnw|
            nc.vector.tensor_scalar(alnw_sb.bitcast(dt.uint32)[:],
                                    lnw_sb.bitcast(dt.uint32)[:],
                                    0x7FFFFFFF, None, AO.bitwise_and)

            # ---------- internal DRAM ----------
            y_partial = dram.tile([T, 2048], dt.float32)
            ss_part = dram.tile([T], dt.float32)
            ss_glob = dram.tile([T], dt.float32)
            pm_part = dram.tile([T], dt.float32)
            pm_glob = dram.tile([T], dt.float32)
            wsum_part = dram.tile([8], dt.float32)
            wsum_glob = dram.tile([8], dt.float32)
            row_bounce = dram.tile([NG, 4, TG], dt.float32)  # sx / cg+cu / al / spare

            # ---------- weight abs-sum stats ----------
            wsum_row = rowp.tile([1, 8], dt.float32, tag="wsum_row")
            for wi, (w_in, nchunk, wcols) in enumerate((
                    (wgT_in, KC, ISH), (wuT_in, KC, ISH), (wdT_in, IC, 2048))):
                acc = smp.tile([128, 1], dt.float32, tag="wacc")
                for c in range(nchunk):
                    for cc in range(wcols // 512):
                        st = wstage.tile([128, 512], dt.float32, tag="wstage")
                        nc.sync.dma_start(st[:], w_in[c * 128:(c + 1) * 128,
                                                      cc * 512:(cc + 1) * 512])
                        red = smp.tile([128, 1], dt.float32, tag="wred")
                        nc.vector.tensor_reduce(red[:], st[:], mybir.AxisListType.X,
                                                AO.add, apply_absolute_value=True)
                        if c == 0 and cc == 0:
                            nc.vector.tensor_copy(acc[:], red[:])
                        else:
                            nc.vector.tensor_tensor(acc[:], acc[:], red[:], AO.add)
                wsum_ps = ps_misc.tile([128, 512], dt.float32, tag="misc_ps")
                nc.tensor.matmul(wsum_ps[0:1, 0:1], ones_col[:], acc[:], start=True,
                                 stop=True)
                nc.scalar.copy(wsum_row[:, wi:wi + 1], wsum_ps[0:1, 0:1])
            nc.vector.memset(wsum_row[:, 3:8], 0.0)
            nc.sync.dma_start(wsum_part.rearrange("(o f) -> o f", o=1)[:], wsum_row[:])
            nc.gpsimd.collective_compute(
                "AllReduce", AO.add, replica_groups=RG,
                ins=[wsum_part.opt()], outs=[wsum_glob.opt()])

            # scl_row: [sw_g, sw_u, sw_d, mg/127, mu/127, md, 0, 0]
            wsg_row = rowp.tile([1, 8], dt.float32, tag="wsg_row")
            nc.sync.dma_start(wsg_row[:], wsum_glob.rearrange("(o f) -> o f", o=1)[:])
            mean_row = rowp.tile([1, 8], dt.float32, tag="mean_row")
            nc.vector.tensor_scalar(mean_row[:, 0:3], wsg_row[:, 0:3],
                                    float(1.0 / (I * H)), EPS, AO.mult, AO.max)
            scl_row = rowp.tile([1, 8], dt.float32, tag="scl_row")
            rw = rowp.tile([1, 8], dt.float32, tag="rw_row")
            nc.vector.reciprocal(rw[:, 0:3], mean_row[:, 0:3])
            nt = rowp.tile([1, 8], dt.float32, tag="nt_row")
            nc.vector.tensor_tensor(nt[:, 0:3], mean_row[:, 0:3], rw[:, 0:3], AO.mult)
            nc.vector.tensor_scalar(nt[:, 0:3], nt[:, 0:3], -1.0, 2.0, AO.mult, AO.add)
            nc.vector.tensor_tensor(scl_row[:, 0:3], rw[:, 0:3], nt[:, 0:3], AO.mult)
            nc.vector.tensor_scalar(scl_row[:, 3:5], mean_row[:, 0:2],
                                    float(1.0 / 127.0), None, AO.mult)
            nc.vector.tensor_copy(scl_row[:, 5:6], mean_row[:, 2:3])
            nc.vector.memset(scl_row[:, 6:8], 0.0)
            wst_ps = ps_misc.tile([128, 512], dt.float32, tag="misc_ps")
            nc.tensor.matmul(wst_ps[:, 0:8], ones_row[:], scl_row[:], start=True,
                             stop=True)
            wstats = constp.tile([128, 8], dt.float32)
            nc.vector.tensor_copy(wstats[:], wst_ps[:, 0:8])

            # ---------- quantize weights to ternary fp8 ----------
            qwg = wqp.tile([128, KC * ISH], dt.float8e4)
            qwu = wqp.tile([128, KC * ISH], dt.float8e4)
            qwd = wqp.tile([128, IC * 2048], dt.float8e4)
            for (w_in, qw, nchunk, wcols, si) in (
                (wgT_in, qwg, KC, ISH, 0), (wuT_in, qwu, KC, ISH, 1),
                (wdT_in, qwd, IC, 2048, 2),
            ):
                for c in range(nchunk):
                    for cc in range(wcols // 512):
                        st = wstage.tile([128, 512], dt.float32, tag="wstage")
                        nc.sync.dma_start(st[:], w_in[c * 128:(c + 1) * 128,
                                                      cc * 512:(cc + 1) * 512])
                        t1 = wstage.tile([128, 512], dt.float32, tag="wq_t1")
                        nc.vector.tensor_scalar(t1[:], st[:], wstats[:, si:si + 1],
                                                MAGIC, AO.mult, AO.add)
                        nc.vector.tensor_scalar(t1[:], t1[:], -MAGIC, 1.0, AO.add,
                                                AO.min)
                        o0 = c * wcols + cc * 512
                        nc.vector.tensor_scalar(qw[:, o0:o0 + 512], t1[:],
                                                -1.0, None, AO.max)

            # ---------- x-quant prepass (emitted interleaved, fills qxT slots) --
            qxT_slots = {}

            def emit_prepass(g):
                tok0 = g * TG
                xmax = smp.tile([128, TG], dt.float32, tag="xmax")
                for kc in range(KC):
                    st = xstage.tile([128, TG], dt.float32, tag="xs")
                    nc.sync.dma_start(st[:], xT_in[kc * 128:(kc + 1) * 128,
                                                   tok0:tok0 + TG])
                    if kc == 0:
                        nc.vector.tensor_scalar(xmax.bitcast(dt.uint32)[:],
                                                st.bitcast(dt.uint32)[:],
                                                0x7FFFFFFF, None, AO.bitwise_and)
                    else:
                        nc.vector.tensor_scalar(st.bitcast(dt.uint32)[:],
                                                st.bitcast(dt.uint32)[:],
                                                0x7FFFFFFF, None, AO.bitwise_and)
                        nc.vector.tensor_tensor(xmax[:], xmax[:], st[:], AO.max)
                mx_nat = smp.tile([128, NTC], dt.float32, tag="mx_nat")
                for c in range(NTC):
                    tr_ps = ps_misc.tile([128, 512], dt.float32, tag="misc_ps")
                    nc.tensor.transpose(tr_ps[:, 0:128],
                                        xmax[:, c * 128:(c + 1) * 128], ident[:])
                    nc.vector.tensor_reduce(mx_nat[:, c:c + 1], tr_ps[:, 0:128],
                                            mybir.AxisListType.X, AO.max)
                nc.vector.tensor_scalar(mx_nat[:], mx_nat[:], EPS, None, AO.max)
                # sx = 127/mxc (reciprocal + newton)
                r0 = smp.tile([128, NTC], dt.float32, tag="sx_r0")
                nc.vector.reciprocal(r0[:], mx_nat[:])
                ntr = smp.tile([128, NTC], dt.float32, tag="sx_nt")
                nc.vector.tensor_tensor(ntr[:], mx_nat[:], r0[:], AO.mult)
                nc.vector.tensor_scalar(ntr[:], ntr[:], -1.0, 2.0, AO.mult, AO.add)
                sxn = smp.tile([128, NTC], dt.float32, tag="sxn")
                nc.vector.tensor_tensor(sxn[:], r0[:], ntr[:], AO.mult)
                nc.vector.tensor_scalar(sxn[:], sxn[:], 127.0, None, AO.mult)
                # cg/cu rows for the main loop, packed as [p, (s c)] s=0:cg 1:cu
                cgcu = smp.tile([128, 2 * NTC], dt.float32, tag="cgcu")
                nc.vector.tensor_scalar(cgcu[:, 0:NTC], mx_nat[:], wstats[:, 3:4],
                                        None, AO.mult)
                nc.vector.tensor_scalar(cgcu[:, NTC:2 * NTC], mx_nat[:],
                                        wstats[:, 4:5], None, AO.mult)
                nc.sync.dma_start(
                    row_bounce[g, 1:3].rearrange("s (c p) -> p s c", p=128)[:],
                    cgcu.rearrange("p (s c) -> p s c", c=NTC)[:])
                # sx broadcast (local to the prepass)
                nc.sync.dma_start(
                    row_bounce[g, 0].rearrange("(c p) -> p c", p=128)[:], sxn[:])
                sx_row = rowp.tile([1, TG], dt.float32, tag="grow")
                nc.sync.dma_start(
                    sx_row[:], row_bounce[g, 0].rearrange("(o f) -> o f", o=1)[:])
                sx_ps = ps_misc.tile([128, 512], dt.float32, tag="misc_ps")
                nc.tensor.matmul(sx_ps[:, 0:TG], ones_row[:], sx_row[:], start=True,
                                 stop=True)
                sx_tile = sxp.tile([128, TG], dt.float32, tag="sx_tile")
                nc.scalar.copy(sx_tile[:], sx_ps[:, 0:TG])
                qxT = qxp.tile([128, KC * TG], dt.bfloat16, tag="qxT")
                qxT_slots[g] = qxT
                for kc in range(KC):
                    st = xstage.tile([128, TG], dt.float32, tag="xs")
                    nc.sync.dma_start(st[:], xT_in[kc * 128:(kc + 1) * 128,
                                                   tok0:tok0 + TG])
                    tq = xstage.tile([128, TG], dt.float32, tag="xq_t")
                    nc.vector.tensor_tensor(tq[:], st[:], sx_tile[:], AO.mult)
                    nc.vector.tensor_scalar(tq[:], tq[:], MAGIC, -MAGIC, AO.add,
                                            AO.add)
                    nc.vector.tensor_scalar(qxT[:, kc * TG:(kc + 1) * TG], tq[:],
                                            127.0, -128.0, AO.min, AO.max)

            # ---------- main pipeline (software-pipelined emission) ----------
            cd_slots = {}
            hT_slots = {}

            def emit_phase1(g):
                tok0 = g * TG
                qxT = qxT_slots.pop(g)
                # cg/cu broadcast tiles
                ccrow = rowp2.tile([1, 2 * TG], dt.float32, tag="grow2")
                nc.sync.dma_start(
                    ccrow[:], row_bounce[g, 1:3]
                    .rearrange("s f -> (s f)").rearrange("(o f) -> o f", o=1)[:])
                cg_tile = bcp.tile([128, TG], dt.float32, tag="cg_tile")
                cu_tile = bcp.tile([128, TG], dt.float32, tag="cu_tile")
                for ri, tile_ in ((0, cg_tile), (1, cu_tile)):
                    bc_ps = ps_misc.tile([128, 512], dt.float32, tag="misc_ps")
                    nc.tensor.matmul(bc_ps[:, 0:TG], ones_row[:],
                                     ccrow[:, ri * TG:(ri + 1) * TG],
                                     start=True, stop=True)
                    nc.scalar.copy(tile_[:], bc_ps[:, 0:TG])

                hT = hbp.tile([128, IC * TG], dt.float32, tag="hT")
                hT_slots[g] = hT
                maxt = smp.tile([128, TG], dt.float32, tag="maxt")
                nc.vector.memset(maxt[:], 0.0)
                ss_ps = ps_ss.tile([1, TG], dt.float32, tag="ss_ps")
                for ic in range(IC):
                    g_ps = ps_gu.tile([128, TG], dt.float32, tag="g_ps")
                    u_ps = ps_gu.tile([128, TG], dt.float32, tag="u_ps")
                    for kc in range(KC):
                        nc.tensor.matmul(
                            g_ps[:],
                            qwg[:, kc * ISH + ic * 128: kc * ISH + (ic + 1) * 128],
                            qxT[:, kc * TG:(kc + 1) * TG],
                            start=(kc == 0), stop=(kc == KC - 1))
                    for kc in range(KC):
                        nc.tensor.matmul(
                            u_ps[:],
                            qwu[:, kc * ISH + ic * 128: kc * ISH + (ic + 1) * 128],
                            qxT[:, kc * TG:(kc + 1) * TG],
                            start=(kc == 0), stop=(kc == KC - 1))
                    gv = evp.tile([128, TG], dt.float32, tag="gsv")
                    nc.vector.tensor_tensor(gv[:], g_ps[:], cg_tile[:], AO.mult)
                    sv = evp.tile([128, TG], dt.float32, tag="gsv")
                    nc.scalar.activation(sv[:], gv[:], AF.Silu)
                    uv = evp.tile([128, TG], dt.float32, tag="uv")
                    nc.vector.tensor_tensor(uv[:], u_ps[:], cu_tile[:], AO.mult)
                    hslice = hT[:, ic * TG:(ic + 1) * TG]
                    nc.vector.tensor_tensor(hslice, sv[:], uv[:], AO.mult)
                    h2 = evp.tile([128, TG], dt.float32, tag="h2")
                    nc.scalar.square(h2[:], hslice)
                    nc.tensor.matmul(ss_ps[:], ones_col[:], h2[:],
                                     start=(ic == 0), stop=(ic == IC - 1))
                    ha = evp.tile([128, TG], dt.float32, tag="h2")
                    nc.scalar.activation(ha[:], hslice, AF.Abs)
                    nc.vector.scalar_tensor_tensor(maxt[:], ha[:],
                                                   alnw_sb[:, ic:ic + 1], maxt[:],
                                                   AO.mult, AO.max)
                pm_nat = smp.tile([128, NTC], dt.float32, tag="pm_nat")
                for c in range(NTC):
                    tr_ps = ps_misc.tile([128, 512], dt.float32, tag="misc_ps")
                    nc.tensor.transpose(tr_ps[:, 0:128],
                                        maxt[:, c * 128:(c + 1) * 128], ident[:])
                    nc.vector.tensor_reduce(pm_nat[:, c:c + 1], tr_ps[:, 0:128],
                                            mybir.AxisListType.X, AO.max)
                ss_row = rowp.tile([1, TG], dt.float32, tag="grow")
                nc.vector.tensor_copy(ss_row[:], ss_ps[:])
                nc.sync.dma_start(ss_part[tok0:tok0 + TG]
                                  .rearrange("(o f) -> o f", o=1)[:], ss_row[:])
                nc.sync.dma_start(pm_part[tok0:tok0 + TG]
                                  .rearrange("(c p) -> p c", p=128)[:], pm_nat[:])
                nc.gpsimd.collective_compute(
                    "AllReduce", AO.add, replica_groups=RG,
                    ins=[ss_part[tok0:tok0 + TG].opt()],
                    outs=[ss_glob[tok0:tok0 + TG].opt()])
                nc.gpsimd.collective_compute(
                    "AllReduce", AO.max, replica_groups=RG,
                    ins=[pm_part[tok0:tok0 + TG].opt()],
                    outs=[pm_glob[tok0:tok0 + TG].opt()])

            def emit_phase2(g):
                tok0 = g * TG
                hT = hT_slots.pop(g)
                # global per-token scales
                ssg = smp.tile([128, NTC], dt.float32, tag="ssg")
                nc.sync.dma_start(ssg[:], ss_glob[tok0:tok0 + TG]
                                  .rearrange("(c p) -> p c", p=128)[:])
                pmg = smp.tile([128, NTC], dt.float32, tag="pmg")
                nc.sync.dma_start(pmg[:], pm_glob[tok0:tok0 + TG]
                                  .rearrange("(c p) -> p c", p=128)[:])
                vr = smp.tile([128, NTC], dt.float32, tag="vr")
                nc.vector.tensor_scalar(vr[:], ssg[:], float(1.0 / I), RMS_EPS,
                                        AO.mult, AO.add)
                sq = smp.tile([128, NTC], dt.float32, tag="sq")
                nc.scalar.sqrt(sq[:], vr[:])
                rr = smp.tile([128, NTC], dt.float32, tag="rr")
                nc.vector.reciprocal(rr[:], sq[:])
                ntn = smp.tile([128, NTC], dt.float32, tag="ntn")
                nc.vector.tensor_tensor(ntn[:], sq[:], rr[:], AO.mult)
                nc.vector.tensor_scalar(ntn[:], ntn[:], -1.0, 2.0, AO.mult, AO.add)
                nc.vector.tensor_tensor(rr[:], rr[:], ntn[:], AO.mult)
                rmc = smp.tile([128, NTC], dt.float32, tag="rmc")
                nc.vector.tensor_tensor(rmc[:], rr[:], pmg[:], AO.mult)
                nc.vector.tensor_scalar(rmc[:], rmc[:], EPS, None, AO.max)
                cd = smp.tile([128, NTC], dt.float32, tag="cd")
                cd_slots[g] = cd
                nc.vector.tensor_scalar(cd[:], rmc[:], wstats[:, 5:6],
                                        float(1.0 / 127.0), AO.mult, AO.mult)
                ar0 = smp.tile([128, NTC], dt.float32, tag="ar0")
                nc.vector.reciprocal(ar0[:], rmc[:])
                ntn2 = smp.tile([128, NTC], dt.float32, tag="ntn2")
                nc.vector.tensor_tensor(ntn2[:], rmc[:], ar0[:], AO.mult)
                nc.vector.tensor_scalar(ntn2[:], ntn2[:], -1.0, 2.0, AO.mult, AO.add)
                nc.vector.tensor_tensor(ar0[:], ar0[:], ntn2[:], AO.mult)
                al_nat = smp.tile([128, NTC], dt.float32, tag="al_nat")
                nc.vector.tensor_tensor(al_nat[:], rr[:], ar0[:], AO.mult)
                nc.vector.tensor_scalar(al_nat[:], al_nat[:], 127.0, None, AO.mult)
                nc.sync.dma_start(
                    row_bounce[g, 3].rearrange("(c p) -> p c", p=128)[:], al_nat[:])
                al_row = rowp.tile([1, TG], dt.float32, tag="grow")
                nc.sync.dma_start(al_row[:],
                                  row_bounce[g, 3].rearrange("(o f) -> o f", o=1)[:])
                alt_ps = ps_misc.tile([128, 512], dt.float32, tag="misc_ps")
                nc.tensor.matmul(alt_ps[:, 0:TG], ones_row[:], al_row[:], start=True,
                                 stop=True)
                al_tile = sxp.tile([128, TG], dt.float32, tag="al_tile")
                nc.scalar.copy(al_tile[:], alt_ps[:, 0:TG])

                # quantize h
                qhT = qhp.tile([128, IC * TG], dt.bfloat16, tag="qhT")
                for ic in range(IC):
                    tq = evp.tile([128, TG], dt.float32, tag="hq_t")
                    nc.vector.scalar_tensor_tensor(tq[:], hT[:, ic * TG:(ic + 1) * TG],
                                                   lnw_sb[:, ic:ic + 1], al_tile[:],
                                                   AO.mult, AO.mult)
                    nc.vector.tensor_scalar(tq[:], tq[:], MAGIC, -MAGIC, AO.add,
                                            AO.add)
                    nc.vector.tensor_scalar(qhT[:, ic * TG:(ic + 1) * TG], tq[:],
                                            127.0, -128.0, AO.min, AO.max)

                # down matmuls + dequant + wide store
                cd = cd_slots.pop(g)
                for tcx in range(NTC):
                    y_row = yrp.tile([128, 2048], dt.float32, tag="y_row")
                    for nh in range(NH):
                        y_ps = ps_dn.tile([128, 512], dt.float32, tag="y_ps")
                        for ic in range(IC):
                            nc.tensor.matmul(
                                y_ps[:],
                                qhT[:, ic * TG + tcx * 128: ic * TG + (tcx + 1) * 128],
                                qwd[:, ic * 2048 + nh * 512: ic * 2048 + (nh + 1) * 512],
                                start=(ic == 0), stop=(ic == IC - 1))
                        nc.scalar.mul(y_row[:, nh * 512:(nh + 1) * 512], y_ps[:],
                                      cd[:, tcx:tcx + 1])
                    nc.sync.dma_start(
                        y_partial[tok0 + tcx * 128: tok0 + (tcx + 1) * 128, :],
                        y_row[:])

                # reduce-scatter every RS_BATCH groups
                if g % RS_BATCH == RS_BATCH - 1:
                    b = g // RS_BATCH
                    rows0 = b * RS_BATCH * TG
                    nrows = RS_BATCH * TG
                    rs_out = dram_rs.tile([nrows // N_CORES, 2048], dt.float32,
                                          tag="rs_out")
                    nc.gpsimd.collective_compute(
                        "ReduceScatter", AO.add, replica_groups=RG,
                        ins=[y_partial[rows0:rows0 + nrows, :].opt()],
                        outs=[rs_out.opt()])
                    nc.sync.dma_start(
                        y_out[b * (nrows // N_CORES):(b + 1) * (nrows // N_CORES), :],
                        rs_out[:])

            # interleaved emission: PE stream stays dense across AR latency
            emit_prepass(0)
            emit_prepass(1)
            for g in range(NG):
                emit_phase1(g)
                if g + 2 < NG:
                    emit_prepass(g + 2)
                if g >= 1:
                    emit_phase2(g - 1)
            emit_phase2(NG - 1)

    nc.compile()
    return nc


def _get_nc():
    if "nc" not in _CACHED:
        _CACHED["nc"] = _build()
    return _CACHED["nc"]


def _make_in_maps(x, w_gate, w_up, w_down, ln_weight):
    xf = np.ascontiguousarray(np.asarray(x, dtype=np.float32).reshape(T, H).T)
    wgT = np.asarray(w_gate, dtype=np.float32).T   # [H, I]
    wuT = np.asarray(w_up, dtype=np.float32).T     # [H, I]
    wdT = np.asarray(w_down, dtype=np.float32).T   # [I, H]
    lnw = np.asarray(ln_weight, dtype=np.float32)
    in_maps = []
    for r in range(N_CORES):
        c0 = r * ISH
        in_maps.append({
            "xT": xf,
            "wgT": np.ascontiguousarray(wgT[:, c0:c0 + ISH]),
            "wuT": np.ascontiguousarray(wuT[:, c0:c0 + ISH]),
            "wdT": np.ascontiguousarray(wdT[c0:c0 + ISH, :]),
            "lnw": np.ascontiguousarray(lnw[c0:c0 + ISH]),
        })
    return in_maps


def _assemble(results):
    out = np.empty((T, 2048), dtype=np.float32)
    rows_per_batch = RS_BATCH * TG // N_CORES          # 128
    for r in range(N_CORES):
        yr = results[r]["y_out"]
        for b in range(NB):
            t0 = b * RS_BATCH * TG + r * rows_per_batch
            out[t0:t0 + rows_per_batch] = \
                yr[b * rows_per_batch:(b + 1) * rows_per_batch]
    return out.reshape(B, S, 2048)


def kernel(x, w_gate, w_up, w_down, ln_weight):
    from concourse import bass_utils

    nc = _get_nc()
    in_maps = _make_in_maps(x, w_gate, w_up, w_down, ln_weight)
    res = bass_utils.run_bass_kernel_spmd(nc, in_maps,
                                          core_ids=list(range(N_CORES)))
    return _assemble(res.results)


# revision 17
# speedup vs baseline: 1.2399x; 1.0049x over previous
"""BitnetMLP on 8 TRN2 NeuronCores — Megatron tensor-parallel over the
intermediate dim I, with exact integer arithmetic on the TensorEngine.

Math: activation fake-quant makes activations exact int8 values and weight
fake-quant makes weights exact ternary values. Both are exactly representable
in bf16/fp8e4, and PSUM accumulates in f32, so every matmul is computed as an
exact integer matmul at full bf16 speed; per-token / per-tensor dequant scales
are applied to the f32 partial sums afterward.

Sharding (per core r of 8):
  w_gate/w_up: I-column shard (1024 of 8192)  -> h^T shard [I_sh=1024, T]
  w_down:      I-row shard                    -> partial y, ReduceScatter(add)
  per-token RMS var and abs-max stats over the full I: AllReduce add / max.

Layouts are feature-major (host pre-transposes x and the weights so the
contract dim lands on SBUF partitions; no on-device transposes of x/w/h).

Structure: an x-quant prepass streams exact-int bf16 x^T tiles to DRAM so the
main per-group matmul pipeline has no latency chains (DRAM gathers / AllReduce
waits overlap matmuls of neighboring groups).
"""
import numpy as np

N_CORES = 8
B, S, H, I = 2, 2048, 2048, 8192
T = B * S                      # 4096 tokens
ISH = I // N_CORES             # 1024  I shard per core
TG = 512                       # tokens per group
NG = T // TG                   # 8 groups
KC = H // 128                  # 16 contract chunks for gate/up
IC = ISH // 128                # 8  contract chunks for down / h^T partition chunks
NH = 2048 // 512               # 4  output col groups for down
NTC = TG // 128                # 4  token tiles per group
RS_BATCH = 2                   # groups per ReduceScatter
NB = NG // RS_BATCH            # 4 RS batches

MAGIC = float(1.5 * 2 ** 23)   # f32 round-to-nearest-even forcing constant
EPS = 1e-5
RMS_EPS = 1e-6

_CACHED = {}


def _build():
    import concourse.bass as bass
    import concourse.bacc as bacc
    import concourse.tile as tile
    import concourse.mybir as mybir
    from concourse import masks
    from contextlib import ExitStack

    dt = mybir.dt
    AO = mybir.AluOpType
    AF = mybir.ActivationFunctionType
    RG = [list(range(N_CORES))]

    nc = bacc.Bacc("TRN2", target_bir_lowering=False, debug=False,
                   num_devices=N_CORES)

    xT_in = nc.dram_tensor("xT", [H, T], dt.float32, kind="ExternalInput")
    wgT_in = nc.dram_tensor("wgT", [H, ISH], dt.float32, kind="ExternalInput")
    wuT_in = nc.dram_tensor("wuT", [H, ISH], dt.float32, kind="ExternalInput")
    wdT_in = nc.dram_tensor("wdT", [ISH, 2048], dt.float32, kind="ExternalInput")
    lnw_in = nc.dram_tensor("lnw", [ISH], dt.float32, kind="ExternalInput")
    y_out = nc.dram_tensor("y_out", [T // N_CORES, 2048], dt.float32,
                           kind="ExternalOutput")

    with tile.TileContext(nc) as tc:
        with ExitStack() as stack:
            ep = stack.enter_context
            constp = ep(tc.tile_pool(name="const", bufs=1))
            wqp = ep(tc.tile_pool(name="wq", bufs=1))
            wstage = ep(tc.tile_pool(name="wstage", bufs=2))
            xstage = ep(tc.tile_pool(name="xstage", bufs=2))
            qxp = ep(tc.tile_pool(name="qx", bufs=2))
            hbp = ep(tc.tile_pool(name="hbuf", bufs=2))
            qhp = ep(tc.tile_pool(name="qh", bufs=2))
            bcp = ep(tc.tile_pool(name="bc", bufs=2))
            sxp = ep(tc.tile_pool(name="sxal", bufs=1))
            yrp = ep(tc.tile_pool(name="yrow", bufs=1))
            smp = ep(tc.tile_pool(name="small", bufs=2))
            rowp = ep(tc.tile_pool(name="rows", bufs=2))
            rowp2 = ep(tc.tile_pool(name="rows2", bufs=1))
            evp = ep(tc.tile_pool(name="evac", bufs=2))
            ps_gu = ep(tc.tile_pool(name="ps_gu", bufs=3, space="PSUM"))
            ps_alt = ep(tc.tile_pool(name="ps_alt", bufs=1, space="PSUM"))
            ps_dn = ep(tc.tile_pool(name="ps_dn", bufs=2, space="PSUM"))
            ps_ss = ep(tc.tile_pool(name="ps_ss", bufs=1, space="PSUM"))
            ps_misc = ep(tc.tile_pool(name="ps_misc", bufs=1, space="PSUM"))
            dram = ep(tc.tile_pool(name="dram", bufs=1, space="DRAM"))
            dram_rs = ep(tc.tile_pool(name="dram_rs", bufs=2, space="DRAM"))

            # ---------- constants ----------
            ident = constp.tile([128, 128], dt.float32)
            masks.make_identity(nc, ident[:])
            ones_col = constp.tile([128, 1], dt.float32)   # lhsT for partition sums
            nc.vector.memset(ones_col[:], 1.0)
            ones_row = constp.tile([1, 128], dt.float32)   # lhsT for K=1 broadcasts
            nc.vector.memset(ones_row[:], 1.0)
            lnw_sb = constp.tile([128, IC], dt.float32)    # lnw[128*ic + p] at [p, ic]
            nc.sync.dma_start(lnw_sb[:], lnw_in.rearrange("(c p) -> p c", p=128)[:])
            alnw_sb = constp.tile([128, IC], dt.float32)   # |lnw|
            nc.vector.tensor_scalar(alnw_sb.bitcast(dt.uint32)[:],
                                    lnw_sb.bitcast(dt.uint32)[:],
                                    0x7FFFFFFF, None, AO.bitwise_and)

            # ---------- internal DRAM ----------
            y_partial = dram.tile([T, 2048], dt.float32)
            stat_in = dram.tile([NG, 2, TG], dt.float32)
            stat_out = dram.tile([NG, 2 * N_CORES, TG], dt.float32)
            wsum_part = dram.tile([8], dt.float32)
            wsum_glob = dram.tile([8], dt.float32)
            row_bounce = dram.tile([NG, 4, TG], dt.float32)  # sx / cg+cu / al / spare

            # ---------- weight abs-sum stats ----------
            wsum_row = rowp.tile([1, 8], dt.float32, tag="wsum_row")
            for wi, (w_in, nchunk, wcols) in enumerate((
                    (wgT_in, KC, ISH), (wuT_in, KC, ISH), (wdT_in, IC, 2048))):
                acc = smp.tile([128, 1], dt.float32, tag="wacc")
                for c in range(nchunk):
                    for cc in range(wcols // 512):
                        st = wstage.tile([128, 512], dt.float32, tag="wstage")
                        nc.sync.dma_start(st[:], w_in[c * 128:(c + 1) * 128,
                                                      cc * 512:(cc + 1) * 512])
                        red = smp.tile([128, 1], dt.float32, tag="wred")
                        nc.vector.tensor_reduce(red[:], st[:], mybir.AxisListType.X,
                                                AO.add, apply_absolute_value=True)
                        if c == 0 and cc == 0:
                            nc.vector.tensor_copy(acc[:], red[:])
                        else:
                            nc.vector.tensor_tensor(acc[:], acc[:], red[:], AO.add)
                wsum_ps = ps_misc.tile([128, 512], dt.float32, tag="misc_ps")
                nc.tensor.matmul(wsum_ps[0:1, 0:1], ones_col[:], acc[:], start=True,
                                 stop=True)
                nc.scalar.copy(wsum_row[:, wi:wi + 1], wsum_ps[0:1, 0:1])
            nc.vector.memset(wsum_row[:, 3:8], 0.0)
            nc.sync.dma_start(wsum_part.rearrange("(o f) -> o f", o=1)[:], wsum_row[:])
            nc.gpsimd.collective_compute(
                "AllReduce", AO.add, replica_groups=RG,
                ins=[wsum_part.opt()], outs=[wsum_glob.opt()])

            # scl_row: [sw_g, sw_u, sw_d, mg/127, mu/127, md, 0, 0]
            wsg_row = rowp.tile([1, 8], dt.float32, tag="wsg_row")
            nc.sync.dma_start(wsg_row[:], wsum_glob.rearrange("(o f) -> o f", o=1)[:])
            mean_row = rowp.tile([1, 8], dt.float32, tag="mean_row")
            nc.vector.tensor_scalar(mean_row[:, 0:3], wsg_row[:, 0:3],
                                    float(1.0 / (I * H)), EPS, AO.mult, AO.max)
            scl_row = rowp.tile([1, 8], dt.float32, tag="scl_row")
            rw = rowp.tile([1, 8], dt.float32, tag="rw_row")
            nc.vector.reciprocal(rw[:, 0:3], mean_row[:, 0:3])
            nt = rowp.tile([1, 8], dt.float32, tag="nt_row")
            nc.vector.tensor_tensor(nt[:, 0:3], mean_row[:, 0:3], rw[:, 0:3], AO.mult)
            nc.vector.tensor_scalar(nt[:, 0:3], nt[:, 0:3], -1.0, 2.0, AO.mult, AO.add)
            nc.vector.tensor_tensor(scl_row[:, 0:3], rw[:, 0:3], nt[:, 0:3], AO.mult)
            nc.vector.tensor_scalar(scl_row[:, 3:5], mean_row[:, 0:2],
                                    float(1.0 / 127.0), None, AO.mult)
            nc.vector.tensor_copy(scl_row[:, 5:6], mean_row[:, 2:3])
            nc.vector.memset(scl_row[:, 6:8], 0.0)
            wst_ps = ps_misc.tile([128, 512], dt.float32, tag="misc_ps")
            nc.tensor.matmul(wst_ps[:, 0:8], ones_row[:], scl_row[:], start=True,
                             stop=True)
            wstats = constp.tile([128, 8], dt.float32)
            nc.vector.tensor_copy(wstats[:], wst_ps[:, 0:8])

            # ---------- quantize weights to ternary fp8 ----------
            qwg = wqp.tile([128, KC * ISH], dt.float8e4)
            qwu = wqp.tile([128, KC * ISH], dt.float8e4)
            qwd = wqp.tile([128, IC * 2048], dt.float8e4)
            for (w_in, qw, nchunk, wcols, si) in (
                (wgT_in, qwg, KC, ISH, 0), (wuT_in, qwu, KC, ISH, 1),
                (wdT_in, qwd, IC, 2048, 2),
            ):
                for c in range(nchunk):
                    for cc in range(wcols // 512):
                        st = wstage.tile([128, 512], dt.float32, tag="wstage")
                        nc.sync.dma_start(st[:], w_in[c * 128:(c + 1) * 128,
                                                      cc * 512:(cc + 1) * 512])
                        t1 = wstage.tile([128, 512], dt.float32, tag="wq_t1")
                        nc.vector.tensor_scalar(t1[:], st[:], wstats[:, si:si + 1],
                                                MAGIC, AO.mult, AO.add)
                        nc.vector.tensor_scalar(t1[:], t1[:], -MAGIC, 1.0, AO.add,
                                                AO.min)
                        o0 = c * wcols + cc * 512
                        nc.vector.tensor_scalar(qw[:, o0:o0 + 512], t1[:],
                                                -1.0, None, AO.max)

            # ---------- x-quant prepass (emitted interleaved, fills qxT slots) --
            qxT_slots = {}

            def emit_prepass(g):
                tok0 = g * TG
                xmax = smp.tile([128, TG], dt.float32, tag="xmax")
                for kc in range(KC):
                    st = xstage.tile([128, TG], dt.float32, tag="xs")
                    nc.sync.dma_start(st[:], xT_in[kc * 128:(kc + 1) * 128,
                                                   tok0:tok0 + TG])
                    if kc == 0:
                        nc.vector.tensor_scalar(xmax.bitcast(dt.uint32)[:],
                                                st.bitcast(dt.uint32)[:],
                                                0x7FFFFFFF, None, AO.bitwise_and)
                    else:
                        nc.vector.tensor_scalar(st.bitcast(dt.uint32)[:],
                                                st.bitcast(dt.uint32)[:],
                                                0x7FFFFFFF, None, AO.bitwise_and)
                        nc.vector.tensor_tensor(xmax[:], xmax[:], st[:], AO.max)
                mx_nat = smp.tile([128, NTC], dt.float32, tag="mx_nat")
                for c in range(NTC):
                    tr_ps = ps_misc.tile([128, 512], dt.float32, tag="misc_ps")
                    nc.tensor.transpose(tr_ps[:, 0:128],
                                        xmax[:, c * 128:(c + 1) * 128], ident[:])
                    nc.vector.tensor_reduce(mx_nat[:, c:c + 1], tr_ps[:, 0:128],
                                            mybir.AxisListType.X, AO.max)
                nc.vector.tensor_scalar(mx_nat[:], mx_nat[:], EPS, None, AO.max)
                # sx = 127/mxc (reciprocal + newton)
                r0 = smp.tile([128, NTC], dt.float32, tag="sx_r0")
                nc.vector.reciprocal(r0[:], mx_nat[:])
                ntr = smp.tile([128, NTC], dt.float32, tag="sx_nt")
                nc.vector.tensor_tensor(ntr[:], mx_nat[:], r0[:], AO.mult)
                nc.vector.tensor_scalar(ntr[:], ntr[:], -1.0, 2.0, AO.mult, AO.add)
                sxn = smp.tile([128, NTC], dt.float32, tag="sxn")
                nc.vector.tensor_tensor(sxn[:], r0[:], ntr[:], AO.mult)
                nc.vector.tensor_scalar(sxn[:], sxn[:], 127.0, None, AO.mult)
                # cg/cu rows for the main loop, packed as [p, (s c)] s=0:cg 1:cu
                cgcu = smp.tile([128, 2 * NTC], dt.float32, tag="cgcu")
                nc.vector.tensor_scalar(cgcu[:, 0:NTC], mx_nat[:], wstats[:, 3:4],
                                        None, AO.mult)
                nc.vector.tensor_scalar(cgcu[:, NTC:2 * NTC], mx_nat[:],
                                        wstats[:, 4:5], None, AO.mult)
                nc.sync.dma_start(
                    row_bounce[g, 1:3].rearrange("s (c p) -> p s c", p=128)[:],
                    cgcu.rearrange("p (s c) -> p s c", c=NTC)[:])
                # sx broadcast (local to the prepass)
                nc.sync.dma_start(
                    row_bounce[g, 0].rearrange("(c p) -> p c", p=128)[:], sxn[:])
                sx_row = rowp.tile([1, TG], dt.float32, tag="grow")
                nc.sync.dma_start(
                    sx_row[:], row_bounce[g, 0].rearrange("(o f) -> o f", o=1)[:])
                sx_ps = ps_misc.tile([128, 512], dt.float32, tag="misc_ps")
                nc.tensor.matmul(sx_ps[:, 0:TG], ones_row[:], sx_row[:], start=True,
                                 stop=True)
                sx_tile = sxp.tile([128, TG], dt.float32, tag="sx_tile")
                nc.scalar.copy(sx_tile[:], sx_ps[:, 0:TG])
                qxT = qxp.tile([128, KC * TG], dt.bfloat16, tag="qxT")
                qxT_slots[g] = qxT
                for kc in range(KC):
                    st = xstage.tile([128, TG], dt.float32, tag="xs")
                    nc.sync.dma_start(st[:], xT_in[kc * 128:(kc + 1) * 128,
                                                   tok0:tok0 + TG])
                    tq = xstage.tile([128, TG], dt.float32, tag="xq_t")
                    nc.vector.tensor_tensor(tq[:], st[:], sx_tile[:], AO.mult)
                    nc.vector.tensor_scalar(tq[:], tq[:], MAGIC, -MAGIC, AO.add,
                                            AO.add)
                    nc.vector.tensor_scalar(qxT[:, kc * TG:(kc + 1) * TG], tq[:],
                                            127.0, -128.0, AO.min, AO.max)

            # ---------- main pipeline (software-pipelined emission) ----------
            cd_slots = {}
            hT_slots = {}

            def emit_phase1(g):
                tok0 = g * TG
                qxT = qxT_slots.pop(g)
                # cg/cu broadcast tiles
                ccrow = rowp2.tile([1, 2 * TG], dt.float32, tag="grow2")
                nc.sync.dma_start(
                    ccrow[:], row_bounce[g, 1:3]
                    .rearrange("s f -> (s f)").rearrange("(o f) -> o f", o=1)[:])
                cg_tile = bcp.tile([128, TG], dt.float32, tag="cg_tile")
                cu_tile = bcp.tile([128, TG], dt.float32, tag="cu_tile")
                for ri, tile_ in ((0, cg_tile), (1, cu_tile)):
                    bc_ps = ps_misc.tile([128, 512], dt.float32, tag="misc_ps")
                    nc.tensor.matmul(bc_ps[:, 0:TG], ones_row[:],
                                     ccrow[:, ri * TG:(ri + 1) * TG],
                                     start=True, stop=True)
                    nc.scalar.copy(tile_[:], bc_ps[:, 0:TG])

                hT = hbp.tile([128, IC * TG], dt.float32, tag="hT")
                hT_slots[g] = hT
                maxt = smp.tile([128, TG], dt.float32, tag="maxt")
                nc.vector.memset(maxt[:], 0.0)
                ss_ps = ps_ss.tile([1, TG], dt.float32, tag="ss_ps")
                for ic in range(IC):
                    g_ps = ps_gu.tile([128, TG], dt.float32, tag="gu_ps")
                    u_ps = ps_gu.tile([128, TG], dt.float32, tag="gu_ps")
                    for kc in range(KC):
                        nc.tensor.matmul(
                            g_ps[:],
                            qwg[:, kc * ISH + ic * 128: kc * ISH + (ic + 1) * 128],
                            qxT[:, kc * TG:(kc + 1) * TG],
                            start=(kc == 0), stop=(kc == KC - 1))
                    for kc in range(KC):
                        nc.tensor.matmul(
                            u_ps[:],
                            qwu[:, kc * ISH + ic * 128: kc * ISH + (ic + 1) * 128],
                            qxT[:, kc * TG:(kc + 1) * TG],
                            start=(kc == 0), stop=(kc == KC - 1))
                    gv = evp.tile([128, TG], dt.float32, tag="gsv")
                    nc.vector.tensor_tensor(gv[:], g_ps[:], cg_tile[:], AO.mult)
                    sv = evp.tile([128, TG], dt.float32, tag="gsv")
                    nc.scalar.activation(sv[:], gv[:], AF.Silu)
                    uv = evp.tile([128, TG], dt.float32, tag="uv")
                    nc.vector.tensor_tensor(uv[:], u_ps[:], cu_tile[:], AO.mult)
                    hslice = hT[:, ic * TG:(ic + 1) * TG]
                    nc.vector.tensor_tensor(hslice, sv[:], uv[:], AO.mult)
                    h2 = evp.tile([128, TG], dt.float32, tag="h2")
                    nc.scalar.square(h2[:], hslice)
                    nc.tensor.matmul(ss_ps[:], ones_col[:], h2[:],
                                     start=(ic == 0), stop=(ic == IC - 1))
                    ha = evp.tile([128, TG], dt.float32, tag="h2")
                    nc.scalar.activation(ha[:], hslice, AF.Abs)
                    nc.vector.scalar_tensor_tensor(maxt[:], ha[:],
                                                   alnw_sb[:, ic:ic + 1], maxt[:],
                                                   AO.mult, AO.max)
                pm_nat = smp.tile([128, NTC], dt.float32, tag="pm_nat")
                for c in range(NTC):
                    tr_ps = ps_misc.tile([128, 512], dt.float32, tag="misc_ps")
                    nc.tensor.transpose(tr_ps[:, 0:128],
                                        maxt[:, c * 128:(c + 1) * 128], ident[:])
                    nc.vector.tensor_reduce(pm_nat[:, c:c + 1], tr_ps[:, 0:128],
                                            mybir.AxisListType.X, AO.max)
                ss_row = rowp.tile([1, TG], dt.float32, tag="grow")
                nc.vector.tensor_copy(ss_row[:], ss_ps[:])
                nc.sync.dma_start(stat_in[g, 0].rearrange("(o f) -> o f", o=1)[:],
                                  ss_row[:])
                nc.sync.dma_start(stat_in[g, 1].rearrange("(c p) -> p c", p=128)[:],
                                  pm_nat[:])
                nc.gpsimd.collective_compute(
                    "AllGather", AO.bypass, replica_groups=RG,
                    ins=[stat_in[g].opt()], outs=[stat_out[g].opt()])

            def emit_phase2(g):
                tok0 = g * TG
                hT = hT_slots.pop(g)
                # global per-token scales: reduce the 16 gathered stat rows
                stat16 = smp.tile([2 * N_CORES, TG], dt.float32, tag="stat16")
                nc.sync.dma_start(stat16[:], stat_out[g])
                ssg = smp.tile([128, NTC], dt.float32, tag="ssg")
                pmg = smp.tile([128, NTC], dt.float32, tag="pmg")
                for c in range(NTC):
                    st_ps = ps_misc.tile([128, 512], dt.float32, tag="misc_ps")
                    nc.tensor.transpose(st_ps[:, 0:2 * N_CORES],
                                        stat16[:, c * 128:(c + 1) * 128],
                                        ident[0:2 * N_CORES, 0:2 * N_CORES])
                    stv = st_ps[:, 0:2 * N_CORES].rearrange("p (a b) -> p b a", b=2)
                    nc.vector.tensor_reduce(ssg[:, c:c + 1], stv[:, 0:1, :],
                                            mybir.AxisListType.X, AO.add)
                    nc.vector.tensor_reduce(pmg[:, c:c + 1], stv[:, 1:2, :],
                                            mybir.AxisListType.X, AO.max)
                vr = smp.tile([128, NTC], dt.float32, tag="vr")
                nc.vector.tensor_scalar(vr[:], ssg[:], float(1.0 / I), RMS_EPS,
                                        AO.mult, AO.add)
                sq = smp.tile([128, NTC], dt.float32, tag="sq")
                nc.scalar.sqrt(sq[:], vr[:])
                rr = smp.tile([128, NTC], dt.float32, tag="rr")
                nc.vector.reciprocal(rr[:], sq[:])
                ntn = smp.tile([128, NTC], dt.float32, tag="ntn")
                nc.vector.tensor_tensor(ntn[:], sq[:], rr[:], AO.mult)
                nc.vector.tensor_scalar(ntn[:], ntn[:], -1.0, 2.0, AO.mult, AO.add)
                nc.vector.tensor_tensor(rr[:], rr[:], ntn[:], AO.mult)
                rmc = smp.tile([128, NTC], dt.float32, tag="rmc")
                nc.vector.tensor_tensor(rmc[:], rr[:], pmg[:], AO.mult)
                nc.vector.tensor_scalar(rmc[:], rmc[:], EPS, None, AO.max)
                cd = smp.tile([128, NTC], dt.float32, tag="cd")
                cd_slots[g] = cd
                nc.vector.tensor_scalar(cd[:], rmc[:], wstats[:, 5:6],
                                        float(1.0 / 127.0), AO.mult, AO.mult)
                ar0 = smp.tile([128, NTC], dt.float32, tag="ar0")
                nc.vector.reciprocal(ar0[:], rmc[:])
                ntn2 = smp.tile([128, NTC], dt.float32, tag="ntn2")
                nc.vector.tensor_tensor(ntn2[:], rmc[:], ar0[:], AO.mult)
                nc.vector.tensor_scalar(ntn2[:], ntn2[:], -1.0, 2.0, AO.mult, AO.add)
                nc.vector.tensor_tensor(ar0[:], ar0[:], ntn2[:], AO.mult)
                al_nat = smp.tile([128, NTC], dt.float32, tag="al_nat")
                nc.vector.tensor_tensor(al_nat[:], rr[:], ar0[:], AO.mult)
                nc.vector.tensor_scalar(al_nat[:], al_nat[:], 127.0, None, AO.mult)
                nc.sync.dma_start(
                    row_bounce[g, 3].rearrange("(c p) -> p c", p=128)[:], al_nat[:])
                al_row = rowp.tile([1, TG], dt.float32, tag="grow")
                nc.sync.dma_start(al_row[:],
                                  row_bounce[g, 3].rearrange("(o f) -> o f", o=1)[:])
                alt_ps = ps_alt.tile([128, 512], dt.float32, tag="alt_ps")
                nc.tensor.matmul(alt_ps[:, 0:TG], ones_row[:], al_row[:], start=True,
                                 stop=True)
                al_tile = sxp.tile([128, TG], dt.float32, tag="al_tile")
                nc.scalar.copy(al_tile[:], alt_ps[:, 0:TG])

                # quantize h
                qhT = qhp.tile([128, IC * TG], dt.bfloat16, tag="qhT")
                for ic in range(IC):
                    tq = evp.tile([128, TG], dt.float32, tag="hq_t")
                    nc.vector.scalar_tensor_tensor(tq[:], hT[:, ic * TG:(ic + 1) * TG],
                                                   lnw_sb[:, ic:ic + 1], al_tile[:],
                                                   AO.mult, AO.mult)
                    nc.vector.tensor_scalar(tq[:], tq[:], MAGIC, -MAGIC, AO.add,
                                            AO.add)
                    nc.vector.tensor_scalar(qhT[:, ic * TG:(ic + 1) * TG], tq[:],
                                            127.0, -128.0, AO.min, AO.max)

                # down matmuls + dequant + wide store
                cd = cd_slots.pop(g)
                for tcx in range(NTC):
                    y_row = yrp.tile([128, 2048], dt.float32, tag="y_row")
                    for nh in range(NH):
                        y_ps = ps_dn.tile([128, 512], dt.float32, tag="y_ps")
                        for ic in range(IC):
                            nc.tensor.matmul(
                                y_ps[:],
                                qhT[:, ic * TG + tcx * 128: ic * TG + (tcx + 1) * 128],
                                qwd[:, ic * 2048 + nh * 512: ic * 2048 + (nh + 1) * 512],
                                start=(ic == 0), stop=(ic == IC - 1))
                        nc.scalar.mul(y_row[:, nh * 512:(nh + 1) * 512], y_ps[:],
                                      cd[:, tcx:tcx + 1])
                    nc.sync.dma_start(
                        y_partial[tok0 + tcx * 128: tok0 + (tcx + 1) * 128, :],
                        y_row[:])

                # reduce-scatter every RS_BATCH groups
                if g % RS_BATCH == RS_BATCH - 1:
                    b = g // RS_BATCH
                    rows0 = b * RS_BATCH * TG
                    nrows = RS_BATCH * TG
                    rs_out = dram_rs.tile([nrows // N_CORES, 2048], dt.float32,
                                          tag="rs_out")
                    nc.gpsimd.collective_compute(
                        "ReduceScatter", AO.add, replica_groups=RG,
                        ins=[y_partial[rows0:rows0 + nrows, :].opt()],
                        outs=[rs_out.opt()])
                    nc.sync.dma_start(
                        y_out[b * (nrows // N_CORES):(b + 1) * (nrows // N_CORES), :],
                        rs_out[:])

            # interleaved emission: PE stream stays dense across AR latency
            emit_prepass(0)
            emit_prepass(1)
            for g in range(NG):
                emit_phase1(g)
                if g + 2 < NG:
                    emit_prepass(g + 2)
                if g >= 1:
                    emit_phase2(g - 1)
            emit_phase2(NG - 1)

    nc.compile()
    return nc


def _get_nc():
    if "nc" not in _CACHED:
        _CACHED["nc"] = _build()
    return _CACHED["nc"]


def _make_in_maps(x, w_gate, w_up, w_down, ln_weight):
    xf = np.ascontiguousarray(np.asarray(x, dtype=np.float32).reshape(T, H).T)
    wgT = np.asarray(w_gate, dtype=np.float32).T   # [H, I]
    wuT = np.asarray(w_up, dtype=np.float32).T     # [H, I]
    wdT = np.asarray(w_down, dtype=np.float32).T   # [I, H]
    lnw = np.asarray(ln_weight, dtype=np.float32)
    in_maps = []
    for r in range(N_CORES):
        c0 = r * ISH
        in_maps.append({
            "xT": xf,
            "wgT": np.ascontiguousarray(wgT[:, c0:c0 + ISH]),
            "wuT": np.ascontiguousarray(wuT[:, c0:c0 + ISH]),
            "wdT": np.ascontiguousarray(wdT[c0:c0 + ISH, :]),
            "lnw": np.ascontiguousarray(lnw[c0:c0 + ISH]),
        })
    return in_maps


def _assemble(results):
    out = np.empty((T, 2048), dtype=np.float32)
    rows_per_batch = RS_BATCH * TG // N_CORES          # 128
    for r in range(N_CORES):
        yr = results[r]["y_out"]
        for b in range(NB):
            t0 = b * RS_BATCH * TG + r * rows_per_batch
            out[t0:t0 + rows_per_batch] = \
                yr[b * rows_per_batch:(b + 1) * rows_per_batch]
    return out.reshape(B, S, 2048)


def kernel(x, w_gate, w_up, w_down, ln_weight):
    from concourse import bass_utils

    nc = _get_nc()
    in_maps = _make_in_maps(x, w_gate, w_up, w_down, ln_weight)
    res = bass_utils.run_bass_kernel_spmd(nc, in_maps,
                                          core_ids=list(range(N_CORES)))
    return _assemble(res.results)
